# revision 1
# baseline (speedup 1.0000x reference)
"""MoDA Vision Transformer forward pass on 8 Trainium2 NeuronCores.

Sharding: pure data-parallel over batch (B=8 -> 1 image per core, weights
replicated, no collectives).

Per-core kernel design (all fp32 storage, float32r matmuls):
- Activations live TRANSPOSED in SBUF: hT[d, t], d=768 over 6 partition-tiles
  of 128, t padded 197->256 so float32r matmuls run at 1 cycle/row.
- Linear layers compute yT = W.T-free matmul: out[F-tile, t] =
  matmul(lhsT=W[k-tile, F-slice], rhs=xT[k-tile, :]), accumulating over 6 (or
  24) k-tiles in PSUM.
- LayerNorm (over d = partitions): sums via ones-matmul (ones scaled 1/768),
  rstd/mean broadcast back across partitions via K=1 ones matmul, applied with
  DVE; gamma/beta are per-partition scalars in this layout.
- Attention (GQA with kv_heads=1, depth-KV cache over layers): scores are
  computed TRANSPOSED, S^T[kpos, q] = matmul(lhsT=KC[64, kpos], rhs=Q'T[64, q])
  where q=(g,t) flattened (12*197=2364). exp on ACT (scale=1/8 folded in),
  softmax denominators come free from a ones-column appended to V:
  O'T[65, q] = matmul(lhsT=V_aug[kpos, 65], rhs=expS^T[kpos, q]); row 64 is
  the denominator. Normalized with a reciprocal + partition-broadcast DMA.
- GELU exact (erf) on ACT while evacuating fc1 PSUM.
"""

import numpy as np

import concourse.bass as bass
import concourse.mybir as mybir
import concourse.tile as tile
from concourse import bacc
from concourse.bass_utils import run_bass_kernel_spmd
from concourse.masks import make_identity

F32 = mybir.dt.float32
F32R = mybir.dt.float32r
AF = mybir.ActivationFunctionType
OP = mybir.AluOpType

B, CIN, IMG, P = 8, 3, 224, 16
DIM, DEPTH, NH, NKV = 768, 12, 12, 1
HD = DIM // NH              # 64
G = NH // NKV               # 12
NPATCH = (IMG // P) ** 2    # 196
T = NPATCH + 1              # 197
TP = 256                    # padded token count (>=256 for f32r full rate)
KD = DIM // 128             # 6 k-tiles
MLP = 4 * DIM               # 3072
MD = MLP // 128             # 24
NQ = G * T                  # 2364 flattened (g, t) query index
NKBLK = (DEPTH * T + 127) // 128  # 19 kpos blocks of 128
SCALE = HD ** -0.5
EPS = 1e-6
NCLS = 1000

# q chunks for attention (all >=256 for f32r full rate)
QCH = [(0, 512), (512, 512), (1024, 512), (1536, 512), (2048, 316)]

_CACHED = {}


def r(ap):
    return ap


def build_module():
    nc = bacc.Bacc("TRN2", target_bir_lowering=False, debug=False, num_devices=8)

    d = {}
    MM_DRAM = {"x", "patch_w", "q_w", "kv_w", "proj_w", "fc1_w", "fc2_w", "head_w"}
    def din(name, shape):
        dt_ = F32R if name in MM_DRAM else F32
        d[name] = nc.dram_tensor(name, shape, dt_, kind="ExternalInput")
    din("x", (CIN, IMG, IMG))
    din("patch_w", (CIN * P * P, DIM)); din("patch_b", (DIM,))
    din("cls_token", (1, 1, DIM)); din("pos_embed", (1, T, DIM))
    for n, sh in [("ln1_w", (DIM,)), ("ln1_b", (DIM,)),
                  ("q_w", (DIM, DIM)), ("q_b", (DIM,)),
                  ("kv_w", (DIM, 2 * HD)), ("kv_b", (2 * HD,)),
                  ("proj_w", (DIM, DIM)), ("proj_b", (DIM,)),
                  ("ln2_w", (DIM,)), ("ln2_b", (DIM,)),
                  ("fc1_w", (DIM, MLP)), ("fc1_b", (MLP,)),
                  ("fc2_w", (MLP, DIM)), ("fc2_b", (DIM,))]:
        din(n, (DEPTH,) + sh)
    din("norm_w", (DIM,)); din("norm_b", (DIM,))
    d["_ones_col"] = nc.dram_tensor("_ones_col", (128,), F32R, kind="ExternalInput")
    d["_ones_row"] = nc.dram_tensor("_ones_row", (128,), F32R, kind="ExternalInput")
    d["_vones"] = nc.dram_tensor("_vones", (NKBLK,), F32R, kind="ExternalInput")
    d["_zeros"] = nc.dram_tensor("_zeros", (KD * TP,), F32R, kind="ExternalInput")
    din("head_w", (DIM, NCLS)); din("head_b", (NCLS,))
    out_d = nc.dram_tensor("out", (1, NCLS), F32, kind="ExternalOutput")

    with tile.TileContext(nc) as tc:
        with (
            tc.tile_pool(name="persist", bufs=1) as persist,
            tc.tile_pool(name="wq", bufs=2) as wq,        # q/proj full matrices
            tc.tile_pool(name="wsl", bufs=2) as wsl,      # fc1/fc2 slabs
            tc.tile_pool(name="vecs", bufs=2) as vecs,    # per-layer bias vectors
            tc.tile_pool(name="act", bufs=1) as actp,     # per-layer activations
            tc.tile_pool(name="tmp", bufs=3) as tmp,      # transient sbuf
            tc.tile_pool(name="ps", bufs=1, space="PSUM") as ps,
            tc.tile_pool(name="dscr", bufs=1, space="DRAM") as dscr,
        ):
            # ---------------- persistent state ----------------
            hT = persist.tile([128, KD, TP], F32R)          # residual stream ^T
            KC = persist.tile([64, DEPTH * T], F32R)        # K cache ^T
            VC = persist.tile([128, NKBLK * (HD + 1)], F32R)  # V cache + ones col
            qpt = persist.tile([64, NQ], F32R)              # Q'^T (g,t)
            otn = persist.tile([64, NQ], F32R)              # normalized O'^T
            ident = persist.tile([128, 128], F32)
            ones_col = persist.tile([128, 1], F32R)         # 1/768 (LN sums)
            ones_row = persist.tile([1, 128], F32R)         # 1.0 (bcast matmul)
            eps_t = persist.tile([1, 1], F32)

            zsrc = d["_zeros"].ap().rearrange("(o c) -> o c", o=1)
            nc.gpsimd.dma_start(hT, zsrc.to_broadcast([128, KD * TP]))
            nc.gpsimd.dma_start(VC, zsrc[:, 0:NKBLK * (HD + 1)].to_broadcast([128, NKBLK * (HD + 1)]))
            make_identity(nc, ident)
            nc.sync.dma_start(ones_col, d["_ones_col"].ap().rearrange("(p o) -> p o", o=1))
            nc.sync.dma_start(ones_row, d["_ones_row"].ap().rearrange("(o p) -> o p", o=1))
            nc.vector.memset(eps_t, EPS)
            # ones column of V_aug (col HD of each 65-wide block)
            nc.gpsimd.dma_start(
                VC[:, HD::(HD + 1)],
                d["_vones"].ap().rearrange("(o c) -> o c", o=1).to_broadcast([128, NKBLK]))

            # ---------------- helpers ----------------
            def load_vec(src_ap, k):
                """DRAM [k*128] vector -> SBUF [128, k] (p, k) layout."""
                t_ = vecs.tile([128, k], F32, name=f"v{load_vec.i}", tag=f"v{load_vec.i % 8}")
                load_vec.i += 1
                nc.sync.dma_start(t_, src_ap.rearrange("(k p) -> p k", p=128))
                return t_
            load_vec.i = 0

            def ln(src, dst, w_sb, b_sb):
                """LayerNorm over d (partitions x k-tiles) of src -> dst.

                src/dst: [128, KD, TP] tiles. w_sb/b_sb: [128, KD]."""
                ssum = ps.tile([1, TP], F32, name="lnsum", tag="row", bufs=2)
                ssq = ps.tile([1, TP], F32, name="lnsq", tag="row", bufs=2)
                for k in range(KD):
                    sq = tmp.tile([128, TP], F32R, name="lnsqt", tag="sq", bufs=2)
                    nc.scalar.activation(sq, src[:, k, :], AF.Square)
                    nc.tensor.matmul(ssum, r(ones_col), r(src[:, k, :]),
                                     start=(k == 0), stop=(k == KD - 1))
                    nc.tensor.matmul(ssq, r(ones_col), r(sq),
                                     start=(k == 0), stop=(k == KD - 1))
                m2 = tmp.tile([1, TP], F32, name="lnm2", tag="row1", bufs=1)
                nc.scalar.activation(m2, ssum, AF.Square)
                var = tmp.tile([1, TP], F32, name="lnvar", tag="row2", bufs=1)
                nc.vector.tensor_tensor(var, ssq, m2, op=OP.subtract)
                rstd = tmp.tile([1, TP], F32R, name="lnrstd", tag="row4", bufs=1)
                nc.scalar.activation(rstd, var, AF.Abs_reciprocal_sqrt, bias=eps_t)
                mr = tmp.tile([1, TP], F32R, name="lnmr", tag="row5", bufs=1)
                nc.vector.tensor_tensor(mr, ssum, rstd, op=OP.mult)
                rstd_b = ps.tile([128, TP], F32, name="lnrb", tag="acc", bufs=4)
                mr_b = ps.tile([128, TP], F32, name="lnmb", tag="acc", bufs=4)
                nc.tensor.matmul(rstd_b, r(ones_row), r(rstd), start=True, stop=True)
                nc.tensor.matmul(mr_b, r(ones_row), r(mr), start=True, stop=True)
                for k in range(KD):
                    tt = tmp.tile([128, TP], F32, name="lnt", tag="sq", bufs=2)
                    nc.vector.tensor_tensor(tt, src[:, k, :], rstd_b, op=OP.mult)
                    nc.vector.tensor_tensor(tt, tt, mr_b, op=OP.subtract)
                    nc.vector.tensor_scalar(
                        out=dst[:, k, :], in0=tt,
                        scalar1=w_sb[:, k:k + 1], scalar2=b_sb[:, k:k + 1],
                        op0=OP.mult, op1=OP.add)

            # ---------------- patch embed ----------------
            # im2col needs a DRAM restage to keep every DMA at <=3 AP dims:
            # scr[c, b, row, j] <- x[c, row, 16j+b], then
            # xpT[(c,a,b), (i,j)] <- scr[c, b, 16i+a, j]
            HG = IMG // P  # 14
            xpT = actp.tile([128, KD, T], F32R, name="xpT", tag="qtx")
            scr = dscr.tile([CIN, P, IMG, HG], F32R, name="scr", tag="scr")
            for c in range(CIN):
                xc = d["x"].ap()[c].rearrange("r (j b) -> b r j", b=P)
                for b in range(P):
                    nc.sync.dma_start(scr[c, b], xc[b])
            for k in range(KD):
                c, ab = k // 2, (k % 2) * 8
                sc = scr[c].rearrange("b (i a) j -> a b i j", a=P)
                for a in range(8):
                    nc.sync.dma_start(
                        xpT[a * 16:(a + 1) * 16, k, 0:NPATCH].rearrange(
                            "p (i j) -> p i j", i=HG),
                        sc[ab + a])

            pw_sb = wq.tile([128, KD, DIM], F32R, name="pw_sb", tag="wq")
            nc.sync.dma_start(pw_sb, d["patch_w"].ap().rearrange("(k p) c -> p k c", p=128))
            # pos_embed for patches (+ patch_b folded in), token layout
            pos_sb = actp.tile([128, 2, DIM], F32, name="pos_sb", tag="g1")
            nc.sync.dma_start(pos_sb[:, 0, :], d["pos_embed"].ap()[0, 1:129])
            nc.sync.dma_start(pos_sb[:68, 1, :], d["pos_embed"].ap()[0, 129:T])
            pbrow = tmp.tile([1, DIM], F32, name="pbrow", tag="pbrow", bufs=1)
            nc.sync.dma_start(pbrow, d["patch_b"].ap().rearrange("(o c) -> o c", o=1))
            pbb = tmp.tile([128, DIM], F32, name="pbb", tag="tok", bufs=1)
            nc.gpsimd.dma_start(
                pbb, d["patch_b"].ap().rearrange("(o c) -> o c", o=1).to_broadcast([128, DIM]))
            for tc_i, tsz in ((0, 128), (1, 68)):
                nc.vector.tensor_tensor(
                    pos_sb[:tsz, tc_i, :], pos_sb[:tsz, tc_i, :],
                    pbb[:tsz, :], op=OP.add)

            for tc_i, tsz in ((0, 128), (1, 68)):
                tok = tmp.tile([128, DIM], F32, name="tok", tag="tok", bufs=1)
                for n in range(2):
                    pp = ps.tile([128, 384], F32, name="pep", tag="acc", bufs=4)
                    for k in range(KD):
                        nc.tensor.matmul(
                            pp[:tsz], r(xpT[:, k, tc_i * 128:tc_i * 128 + tsz]),
                            r(pw_sb[:, k, n * 384:(n + 1) * 384]),
                            start=(k == 0), stop=(k == KD - 1))
                    nc.vector.tensor_tensor(
                        tok[:tsz, n * 384:(n + 1) * 384], pp[:tsz],
                        pos_sb[:tsz, tc_i, n * 384:(n + 1) * 384], op=OP.add)
                for k in range(KD):
                    tp = ps.tile([128, 128], F32, name="pet", tag="acc", bufs=4)
                    nc.tensor.matmul(tp[:, :tsz], tok[:tsz, k * 128:(k + 1) * 128],
                                     ident[:tsz, :tsz], is_transpose=True,
                                     start=True, stop=True)
                    nc.vector.tensor_copy(
                        hT[:, k, 1 + tc_i * 128:1 + tc_i * 128 + tsz], tp[:, :tsz])

            # cls token (+ pos_embed[0]) -> hT[:, :, 0]
            clsr = tmp.tile([1, DIM], F32, name="clsr", tag="clsr", bufs=1)
            nc.sync.dma_start(clsr, d["cls_token"].ap()[0])
            p0r = tmp.tile([1, DIM], F32, name="p0r", tag="p0r", bufs=1)
            nc.sync.dma_start(p0r, d["pos_embed"].ap()[0, 0:1])
            nc.vector.tensor_tensor(clsr, clsr, p0r, op=OP.add)
            csd = dscr.tile([DIM], F32R, name="csd", tag="csd")
            nc.sync.dma_start(csd.rearrange("(o c) -> o c", o=1), clsr.bitcast(F32R))
            nc.sync.dma_start(hT[:, :, 0:1],
                              csd.rearrange("(k p) -> p k", p=128))

            # ---------------- transformer layers ----------------
            hnT = persist.tile([128, KD, TP], F32R)
            g1T = actp.tile([128, MD, TP], F32R, name="g1T", tag="g1")
            oT = persist.tile([128, KD, TP], F32R)
            nc.gpsimd.dma_start(oT, zsrc.to_broadcast([128, KD * TP]))
            qt_sb = actp.tile([128, KD, T], F32R, name="qt_sb", tag="qtx")

            for l in range(DEPTH):
                ln1w = load_vec(d["ln1_w"].ap()[l], KD)
                ln1b = load_vec(d["ln1_b"].ap()[l], KD)
                # prefetch MLP weights early (slabs stream during attention)
                f1r = d["fc1_w"].ap()[l].rearrange("(k p) c -> p k c", p=128)
                f2r = d["fc2_w"].ap()[l].rearrange("(k p) c -> p k c", p=128)
                f1tiles = []
                for mq in range(6):  # 512 F-cols each (2KB segments)
                    f1s = wsl.tile([128, KD, 512], F32R, name="f1s", tag="f1s", bufs=3)
                    nc.sync.dma_start(f1s, f1r[:, :, mq * 512:(mq + 1) * 512])
                    f1tiles.append(f1s)
                f2tiles = []
                for kc in range(6):  # 4 k-tiles each, full 768 cols (3KB segments)
                    f2q = wsl.tile([128, 4, DIM], F32R, name="f2q", tag="f1s", bufs=3)
                    nc.sync.dma_start(f2q, f2r[:, kc * 4:(kc + 1) * 4, :])
                    f2tiles.append(f2q)
                ln(hT, hnT, ln1w, ln1b)

                # ---- Q projection -> qt_sb (QT layout), then Q'T strips ----
                qw_sb = wq.tile([128, KD, DIM], F32R, name="qw", tag="wq")
                nc.sync.dma_start(qw_sb, d["q_w"].ap()[l].rearrange("(k p) c -> p k c", p=128))
                qb = load_vec(d["q_b"].ap()[l], KD)
                for m in range(KD):
                    qp = ps.tile([128, TP], F32, name="qtp", tag="acc", bufs=4)
                    for k in range(KD):
                        nc.tensor.matmul(qp, r(qw_sb[:, k, m * 128:(m + 1) * 128]),
                                         r(hnT[:, k, :]),
                                         start=(k == 0), stop=(k == KD - 1))
                    nc.vector.tensor_scalar(out=qt_sb[:, m, :], in0=qp[:, 0:T],
                                            scalar1=qb[:, m:m + 1], scalar2=None,
                                            op0=OP.add)
                for g in range(G):
                    j, half = g // 2, g % 2
                    if half == 0:
                        nc.vector.tensor_copy(qpt[:, g * T:(g + 1) * T],
                                              qt_sb[0:64, j, :])
                    else:
                        nc.sync.dma_start(qpt[:, g * T:(g + 1) * T],
                                          qt_sb[64:128, j, :])

                # ---- KV projection; append K^T and V to caches ----
                kvw = wq.tile([128, KD, 2 * HD], F32R, name="kvw", tag="wkv", bufs=1)
                nc.sync.dma_start(kvw, d["kv_w"].ap()[l].rearrange("(k p) c -> p k c", p=128))
                kb = vecs.tile([64, 1], F32, name=f"kb{l}", tag="kb")
                vb = vecs.tile([64, 1], F32, name=f"vb{l}", tag="vb")
                nc.sync.dma_start(kb, d["kv_b"].ap()[l, 0:HD].rearrange("(p o) -> p o", o=1))
                nc.sync.dma_start(vb, d["kv_b"].ap()[l, HD:2 * HD].rearrange("(p o) -> p o", o=1))
                kp = ps.tile([64, TP], F32, name="kp", tag="acc", bufs=4)
                vp = ps.tile([64, TP], F32, name="vp", tag="acc", bufs=4)
                for k in range(KD):
                    nc.tensor.matmul(kp, r(kvw[:, k, 0:HD]), r(hnT[:, k, :]),
                                     start=(k == 0), stop=(k == KD - 1))
                for k in range(KD):
                    nc.tensor.matmul(vp, r(kvw[:, k, HD:2 * HD]), r(hnT[:, k, :]),
                                     start=(k == 0), stop=(k == KD - 1))
                nc.vector.tensor_scalar(out=KC[:, l * T:(l + 1) * T], in0=kp[:, 0:T],
                                        scalar1=kb, scalar2=None, op0=OP.add)
                vsb = tmp.tile([64, TP], F32, name="vsb", tag="vsb", bufs=1)
                nc.vector.tensor_scalar(out=vsb, in0=vp, scalar1=vb, scalar2=None,
                                        op0=OP.add)
                # transpose v^T [64, t] -> v [t, 64] and scatter into VC
                for tc_i, tsz in ((0, 128), (1, 69)):
                    vtp = ps.tile([128, HD], F32, name="vtp", tag="acc", bufs=4)
                    nc.tensor.matmul(vtp[:tsz], vsb[:, tc_i * 128:tc_i * 128 + tsz],
                                     ident[0:64, 0:HD], is_transpose=True,
                                     start=True, stop=True)
                    vts = tmp.tile([128, HD], F32R, name="vts", tag="vts", bufs=2)
                    nc.vector.tensor_copy(vts[:tsz], vtp[:tsz])
                    # rows t0..t0+tsz land at kpos = l*T + tc_i*128 + [0, tsz)
                    t0 = 0
                    while t0 < tsz:
                        kpos = l * T + tc_i * 128 + t0
                        blk, off = kpos // 128, kpos % 128
                        cnt = min(tsz - t0, 128 - off)
                        nc.sync.dma_start(
                            VC[off:off + cnt, blk * (HD + 1):blk * (HD + 1) + HD],
                            vts[t0:t0 + cnt, :])
                        t0 += cnt

                # ---- attention ----
                Lk = (l + 1) * T
                nkt = (Lk + 127) // 128
                for qoff, qsz in QCH:
                    ot = ps.tile([65, 512], F32, name="otp", tag="ot", bufs=2)
                    for c in range(nkt):
                        ksz = min(128, Lk - c * 128)
                        st = ps.tile([128, 512], F32, name="stp", tag="acc", bufs=4)
                        nc.tensor.matmul(st[:ksz, :qsz],
                                         r(KC[:, c * 128:c * 128 + ksz]),
                                         r(qpt[:, qoff:qoff + qsz]),
                                         start=True, stop=True)
                        pt = tmp.tile([128, 512], F32R, name="pt", tag="pt")
                        nc.scalar.activation(pt[:ksz, :qsz], st[:ksz, :qsz],
                                             AF.Exp, scale=SCALE)
                        nc.tensor.matmul(ot[:, :qsz],
                                         r(VC[0:ksz, c * (HD + 1):(c + 1) * (HD + 1)]),
                                         r(pt[:ksz, :qsz]),
                                         start=(c == 0), stop=(c == nkt - 1))
                    # evacuate PSUM fast (frees the ot bank), then normalize in SBUF
                    ots = tmp.tile([65, 512], F32, name="ots", tag="ots", bufs=2)
                    nc.vector.tensor_copy(ots[:, :qsz], ot[:, :qsz])
                    ri = tmp.tile([1, 512], F32, name="ri", tag="ri", bufs=1)
                    nc.scalar.activation(ri[:, :qsz], ots[64:65, :qsz],
                                         AF.Abs_reciprocal_sqrt)
                    rs = tmp.tile([1, 512], F32R, name="rs", tag="rs", bufs=1)
                    nc.scalar.activation(rs[:, :qsz], ri[:, :qsz], AF.Square)
                    rbp = ps.tile([64, 512], F32, name="rbp", tag="acc", bufs=4)
                    nc.tensor.matmul(rbp[:, :qsz], r(ones_row[:, 0:64]), r(rs[:, :qsz]),
                                     start=True, stop=True)
                    rb = tmp.tile([64, 512], F32, name="rb", tag="rb", bufs=1)
                    nc.vector.tensor_copy(rb[:, :qsz], rbp[:, :qsz])
                    nc.vector.tensor_tensor(otn[:, qoff:qoff + qsz],
                                            ots[0:64, :qsz], rb[:, :qsz], op=OP.mult)

                # ---- reshape O'T (g,t) -> oT [d, t] ----
                for g in range(G):
                    j, half = g // 2, g % 2
                    if half == 0:
                        nc.vector.tensor_copy(oT[0:64, j, 0:T], otn[:, g * T:(g + 1) * T])
                    else:
                        nc.sync.dma_start(oT[64:128, j, 0:T], otn[:, g * T:(g + 1) * T])

                # ---- output projection + residual ----
                ow_sb = wq.tile([128, KD, DIM], F32R, name="ow", tag="wq")
                nc.sync.dma_start(ow_sb, d["proj_w"].ap()[l].rearrange("(k p) c -> p k c", p=128))
                ob = load_vec(d["proj_b"].ap()[l], KD)
                for m in range(KD):
                    op_ = ps.tile([128, TP], F32, name="prp", tag="acc", bufs=4)
                    for k in range(KD):
                        nc.tensor.matmul(op_, r(ow_sb[:, k, m * 128:(m + 1) * 128]),
                                         r(oT[:, k, :]),
                                         start=(k == 0), stop=(k == KD - 1))
                    nc.vector.scalar_tensor_tensor(
                        out=hT[:, m, 0:T], in0=op_[:, 0:T], scalar=ob[:, m:m + 1],
                        in1=hT[:, m, 0:T], op0=OP.add, op1=OP.add)

                # ---- MLP ----
                ln2w = load_vec(d["ln2_w"].ap()[l], KD)
                ln2b = load_vec(d["ln2_b"].ap()[l], KD)
                ln(hT, hnT, ln2w, ln2b)
                f1b = load_vec(d["fc1_b"].ap()[l], MD)
                for mq in range(6):
                    f1s = f1tiles[mq]
                    for mi in range(4):
                        m = mq * 4 + mi
                        fp = ps.tile([128, TP], F32, name="f1p", tag="acc", bufs=4)
                        for k in range(KD):
                            nc.tensor.matmul(fp, r(f1s[:, k, mi * 128:(mi + 1) * 128]),
                                             r(hnT[:, k, :]),
                                             start=(k == 0), stop=(k == KD - 1))
                        nc.scalar.activation(g1T[:, m, :], fp, AF.Gelu,
                                             bias=f1b[:, m:m + 1])
                f2b = load_vec(d["fc2_b"].ap()[l], KD)
                f2acc = [ps.tile([128, TP], F32, name=f"f2p{m}",
                                 tag=("acc" if m < 4 else "ot"), bufs=(4 if m < 4 else 2))
                         for m in range(KD)]
                for kc in range(6):
                    f2q = f2tiles[kc]
                    for kk in range(4):
                        k = kc * 4 + kk
                        for m in range(KD):
                            nc.tensor.matmul(f2acc[m], r(f2q[:, kk, m * 128:(m + 1) * 128]),
                                             r(g1T[:, k, :]),
                                             start=(k == 0), stop=(k == MD - 1))
                for m in range(KD):
                    nc.vector.scalar_tensor_tensor(
                        out=hT[:, m, 0:T], in0=f2acc[m][:, 0:T], scalar=f2b[:, m:m + 1],
                        in1=hT[:, m, 0:T], op0=OP.add, op1=OP.add)

            # ---------------- final LN + head ----------------
            nw = load_vec(d["norm_w"].ap(), KD)
            nb = load_vec(d["norm_b"].ap(), KD)
            ln(hT, hnT, nw, nb)
            orow = persist.tile([1, NCLS], F32)
            nc.sync.dma_start(orow, d["head_b"].ap().rearrange("(o c) -> o c", o=1))
            hwr = d["head_w"].ap().rearrange("(k p) c -> p k c", p=128)
            for n in range(2):
                hw_c = wsl.tile([128, KD, 500], F32R, name="hw_c", tag="f1s", bufs=3)
                nc.sync.dma_start(hw_c, hwr[:, :, n * 500:(n + 1) * 500])
                hp = ps.tile([1, 500], F32, name="hp", tag="row", bufs=2)
                for k in range(KD):
                    nc.tensor.matmul(hp, r(hnT[:, k, 0:1]),
                                     r(hw_c[:, k, :]),
                                     start=(k == 0), stop=(k == KD - 1))
                nc.vector.tensor_tensor(orow[:, n * 500:(n + 1) * 500], hp,
                                        orow[:, n * 500:(n + 1) * 500], op=OP.add)
            nc.sync.dma_start(out_d.ap(), orow)

    nc.compile()
    return nc


def make_in_maps(inputs):
    names = ["patch_w", "patch_b", "cls_token", "pos_embed", "ln1_w", "ln1_b",
             "q_w", "q_b", "kv_w", "kv_b", "proj_w", "proj_b", "ln2_w", "ln2_b",
             "fc1_w", "fc1_b", "fc2_w", "fc2_b", "norm_w", "norm_b",
             "head_w", "head_b"]
    shared = {n: np.ascontiguousarray(np.asarray(inputs[n], dtype=np.float32))
              for n in names}
    shared["_ones_col"] = np.full((128,), 1.0 / DIM, dtype=np.float32)
    shared["_ones_row"] = np.ones((128,), dtype=np.float32)
    shared["_vones"] = np.ones((NKBLK,), dtype=np.float32)
    shared["_zeros"] = np.zeros((KD * TP,), dtype=np.float32)
    x = np.asarray(inputs["x"], dtype=np.float32)
    return [dict(shared, x=np.ascontiguousarray(x[b])) for b in range(B)]


def kernel(**inputs):
    if "nc" not in _CACHED:
        _CACHED["nc"] = build_module()
    nc = _CACHED["nc"]
    res = run_bass_kernel_spmd(nc, make_in_maps(inputs), core_ids=list(range(B)))
    return np.concatenate([res.results[b]["out"] for b in range(B)], axis=0)



# revision 21
# speedup vs baseline: 1.6079x; 1.6079x over previous
"""MoDA Vision Transformer forward pass on 8 Trainium2 NeuronCores.

Sharding: pure data-parallel over batch (B=8 -> 1 image per core, weights
replicated, no collectives).

v2 design (bf16 compute, fp32 residual):
- All weights pre-tiled + cast to bf16 on HOST into exactly the SBUF layout
  [p, k, c], so every weight DMA is one contiguous chunk per partition
  (128 descriptors per load instead of ~10k strided ones).
- Residual stream hT stays fp32 (f32r) padded to 256 token cols so the
  LayerNorm sum matmuls run at full f32r rate; all other activations are
  bf16 at 208 token cols (bf16 matmuls are full rate at any width).
- im2col + pos_embed/cls/bias folding done on host; patch embed is a
  plain linear directly into the transposed residual layout.
- Attention (GQA kv_heads=1, depth-KV cache): q pieces of (1024,1024,316)
  flattened (g,t) queries; per kpos-block one 1024-wide exp on ACT
  (amortizes the 352-cycle ACTIVATE overhead); denominators via a ones
  column appended to V; softmax normalize with DVE reciprocal.
- PSUM: 3x [128,1024] "big" slots (6 banks) + 2x [1-128,512] "row" slots
  (2 banks) = exactly 8 banks.
- Next layer's weights are prefetched at the top of each layer body on the
  sync HWDGE ring; small strip DMAs ride the scalar HWDGE ring.
"""

import numpy as np
import ml_dtypes

import concourse.bass as bass
import concourse.mybir as mybir
import concourse.tile as tile
from concourse import bacc
from concourse.bass_utils import run_bass_kernel_spmd
from concourse.masks import make_identity

F32 = mybir.dt.float32
F32R = mybir.dt.float32r
BF16 = mybir.dt.bfloat16
AF = mybir.ActivationFunctionType
OP = mybir.AluOpType
BFNP = ml_dtypes.bfloat16

B, CIN, IMG, P = 8, 3, 224, 16
DIM, DEPTH, NH, NKV = 768, 12, 12, 1
HD = DIM // NH              # 64
G = NH // NKV               # 12
NPATCH = (IMG // P) ** 2    # 196
T = NPATCH + 1              # 197
TC = 208                    # bf16 activation token cols (197 padded)
TR = 256                    # fp32 residual token cols (f32r full-rate >=256)
KD = DIM // 128             # 6
MLP = 4 * DIM               # 3072
MD = MLP // 128             # 24
NQ = G * T                  # 2364
NQP = 2368                  # padded
NKBLK = (DEPTH * T + 127) // 128  # 19
VW = HD + 2                 # 66: V block width (64 V + 1 ones + 1 pad)
SCALE = HD ** -0.5
EPS = 1e-6
NCLS = 1000
QP_ = [(0, 1024), (1024, 1024), (2048, NQ - 2048)]  # q pieces

# packed per-layer vector columns (fp32): [128, NV]
VO_L1W, VO_L1B, VO_QB, VO_KVB, VO_PB = 0, 6, 12, 18, 19
VO_L2W, VO_L2B, VO_F1B, VO_F2B, NV = 25, 31, 37, 61, 67

_CACHED = {}


def build_module():
    nc = bacc.Bacc("TRN2", target_bir_lowering=False, debug=False, num_devices=8)

    d = {}
    def din(name, shape, dt_):
        d[name] = nc.dram_tensor(name, shape, dt_, kind="ExternalInput")

    din("xpt", (128, KD * TC), BF16)
    din("posbt", (128, KD * TC), F32)
    din("patchw", (128, KD * DIM), BF16)
    din("qw", (DEPTH, 128, KD * DIM), BF16)
    din("kvw", (DEPTH, 128, KD * 2 * HD), BF16)
    din("projw", (DEPTH, 128, KD * DIM), BF16)
    din("fc1w", (DEPTH, 128, KD * MLP), BF16)
    din("fc2w", (DEPTH, 128, MD * DIM), BF16)
    din("vecs", (DEPTH, 128, NV), F32)
    din("normv", (128, 12), F32)
    din("headw", (128, KD * NCLS), BF16)
    din("headb", (NCLS,), F32)
    din("_ones", (128,), F32R)
    din("_zeros", (KD * TR,), F32R)
    out_d = nc.dram_tensor("out", (1, NCLS), F32, kind="ExternalOutput")

    with tile.TileContext(nc) as tc:
        with (
            tc.tile_pool(name="persist", bufs=1) as persist,
            tc.tile_pool(name="wq", bufs=2) as wq,        # q/kv/proj + vecs
            tc.tile_pool(name="wsl", bufs=10) as wsl,     # fc1/fc2 quarter slabs
            tc.tile_pool(name="tmp", bufs=2) as tmp,      # transient sbuf
            tc.tile_pool(name="ps", bufs=1, space="PSUM") as ps,
        ):
            # ---------------- persistent state ----------------
            hT = persist.tile([128, KD, TR], F32R)          # residual ^T (fp32)
            sqT = persist.tile([128, KD, TR], BF16)         # squares scratch
            hnT = persist.tile([128, KD, TC], BF16)         # LN output ^T
            oT = persist.tile([128, KD, TC], BF16)          # attn out ^T
            g1T = persist.tile([128, MD, TC], BF16)         # gelu(fc1) ^T
            KC = persist.tile([64, NQP], BF16)              # K cache ^T
            VC = persist.tile([128, NKBLK, VW], BF16)       # V cache + ones col
            qpt = persist.tile([64, NQP], BF16)             # Q'^T (g,t)
            otn = persist.tile([64, NQP], BF16)             # normalized O'^T
            ident = persist.tile([128, 128], F32)
            onec = persist.tile([128, 1], F32R)             # 1.0 col (LN sum lhsT)
            onecb = persist.tile([128, 1], BF16)
            oner = persist.tile([1, 128], F32R)             # 1.0 row (bcast lhsT)
            eps_t = persist.tile([1, 1], F32)
            orow = persist.tile([1, NCLS], F32)

            nc.gpsimd.dma_start(
                hT, d["_zeros"].ap().rearrange("(o c) -> o c", o=1)
                .to_broadcast([128, KD * TR]))
            nc.vector.memset(oT, 0.0)
            make_identity(nc, ident)
            nc.sync.dma_start(onec, d["_ones"].ap().rearrange("(p o) -> p o", o=1))
            nc.sync.dma_start(oner, d["_ones"].ap().rearrange("(o p) -> o p", o=1))
            nc.vector.memset(onecb, 1.0)
            nc.vector.memset(eps_t, EPS)
            nc.vector.memset(VC[:, :, HD:HD + 2], 1.0)

            # ---------------- weight loading helpers ----------------
            def load_qkvp(l):
                v = wq.tile([128, NV], F32, name="vecs", tag="vecs")
                nc.sync.dma_start(v, d["vecs"].ap()[l])
                qw_ = wq.tile([128, KD, DIM], BF16, name="qw", tag="qw")
                nc.sync.dma_start(qw_, d["qw"].ap()[l].rearrange("p (k c) -> p k c", k=KD))
                kvw_ = wq.tile([128, KD, 2 * HD], BF16, name="kvw", tag="kvw")
                nc.sync.dma_start(kvw_, d["kvw"].ap()[l].rearrange("p (k c) -> p k c", k=KD))
                ow_ = wq.tile([128, KD, DIM], BF16, name="ow", tag="ow")
                nc.sync.dma_start(ow_, d["projw"].ap()[l].rearrange("p (k c) -> p k c", k=KD))
                return v, qw_, kvw_, ow_

            def load_slabs(l):
                f1r = d["fc1w"].ap()[l].rearrange("p (k c) -> p k c", k=KD)
                f2r = d["fc2w"].ap()[l].rearrange("p (k c) -> p k c", k=MD)
                f1q, f2q = [], []
                for i in range(4):  # fc1 quarter: m-tiles 6i/4.. (768 cols each)
                    s = wsl.tile([128, KD, MLP // 4], BF16, name="f1q", tag="slab")
                    nc.sync.dma_start(s, f1r[:, :, i * (MLP // 4):(i + 1) * (MLP // 4)])
                    f1q.append(s)
                for i in range(4):  # fc2 quarter: k-tiles 6i..6i+5 (full 768 cols)
                    s = wsl.tile([128, KD, DIM], BF16, name="f2q", tag="slab")
                    nc.sync.dma_start(s, f2r[:, i * KD:(i + 1) * KD, :])
                    f2q.append(s)
                return f1q, f2q

            # ---------------- layernorm ----------------
            def ln(dst, wb, wo, bo):
                """LN over d of hT -> dst[128, KD, TC] (bf16).

                wb: [128, NV]-style tile; wo/bo: col offsets of gamma/beta."""
                nc.vector.tensor_tensor(sqT, hT, hT, op=OP.mult)
                ssum = ps.tile([1, 512], F32, name="ssum", tag="row", bufs=2)
                ssq = ps.tile([1, 512], F32, name="ssq", tag="row", bufs=2)
                for k in range(KD):
                    nc.tensor.matmul(ssum[:, 0:TR], onec, hT[:, k, :],
                                     start=(k == 0), stop=(k == KD - 1))
                for k in range(KD):
                    nc.tensor.matmul(ssq[:, 0:TR], onecb, sqT[:, k, :],
                                     start=(k == 0), stop=(k == KD - 1))
                mean = tmp.tile([1, TC], F32, name="mean", tag="mean", bufs=1)
                nc.vector.tensor_scalar(out=mean, in0=ssum[:, 0:TC],
                                        scalar1=1.0 / DIM, scalar2=None, op0=OP.mult)
                m2 = tmp.tile([1, TC], F32, name="m2", tag="m2", bufs=1)
                nc.vector.tensor_tensor(m2, mean, mean, op=OP.mult)
                var = tmp.tile([1, TC], F32, name="var", tag="var", bufs=1)
                nc.vector.scalar_tensor_tensor(
                    out=var, in0=ssq[:, 0:TC], scalar=1.0 / DIM,
                    in1=m2, op0=OP.mult, op1=OP.subtract)
                rstd = tmp.tile([1, TC], F32R, name="rstd", tag="rstd", bufs=1)
                nc.scalar.activation(rstd, var, AF.Abs_reciprocal_sqrt, bias=eps_t)
                mr = tmp.tile([1, TC], F32R, name="mr", tag="mr", bufs=1)
                nc.vector.tensor_tensor(mr, mean, rstd, op=OP.mult)
                rstd_b = ps.tile([128, TC], F32, name="rstd_b", tag="row", bufs=2)
                mr_b = ps.tile([128, TC], F32, name="mr_b", tag="row", bufs=2)
                nc.tensor.matmul(rstd_b, oner, rstd, start=True, stop=True)
                nc.tensor.matmul(mr_b, oner, mr, start=True, stop=True)
                for k in range(KD):
                    t1 = tmp.tile([128, TC], F32, name="lnt", tag="lnt", bufs=2)
                    nc.vector.tensor_tensor(t1, hT[:, k, 0:TC], rstd_b, op=OP.mult)
                    nc.vector.tensor_tensor(t1, t1, mr_b, op=OP.subtract)
                    nc.vector.tensor_scalar(
                        out=dst[:, k, :], in0=t1,
                        scalar1=wb[:, wo + k:wo + k + 1],
                        scalar2=wb[:, bo + k:bo + k + 1],
                        op0=OP.mult, op1=OP.add)

            # ---------------- prologue: patch embed ----------------
            xpt = tmp.tile([128, KD, TC], BF16, name="xpt", tag="pt", bufs=2)
            nc.sync.dma_start(xpt, d["xpt"].ap().rearrange("p (k c) -> p k c", k=KD))
            posbt = tmp.tile([128, KD, TC], F32, name="posbt", tag="posbt", bufs=1)
            nc.sync.dma_start(posbt, d["posbt"].ap().rearrange("p (k c) -> p k c", k=KD))
            pw_sb = wq.tile([128, KD, DIM], BF16, name="pw_sb", tag="qw")
            nc.sync.dma_start(pw_sb, d["patchw"].ap().rearrange("p (k c) -> p k c", k=KD))
            vecs0 = load_qkvp(0)
            slabs0 = load_slabs(0)

            for m in range(KD):
                pp = ps.tile([128, 1024], F32, name="pp", tag="big", bufs=3)
                for k in range(KD):
                    nc.tensor.matmul(pp[:, 0:TC], pw_sb[:, k, m * 128:(m + 1) * 128],
                                     xpt[:, k, :], start=(k == 0), stop=(k == KD - 1))
                nc.vector.tensor_tensor(hT[:, m, 0:TC], pp[:, 0:TC],
                                        posbt[:, m, :], op=OP.add)

            # ---------------- transformer layers ----------------
            lw = (vecs0, slabs0)
            for l in range(DEPTH):
                (vv, qw_sb, kvw_sb, ow_sb), (f1q, f2q) = lw
                if l + 1 < DEPTH:
                    nxt = (load_qkvp(l + 1), load_slabs(l + 1))

                ln(hnT, vv, VO_L1W, VO_L1B)

                # ---- Q projection -> qpt strips ----
                for m in range(KD):
                    qp = ps.tile([128, 1024], F32, name="qp", tag="big", bufs=3)
                    for k in range(KD):
                        nc.tensor.matmul(qp[:, 0:TC], qw_sb[:, k, m * 128:(m + 1) * 128],
                                         hnT[:, k, :], start=(k == 0), stop=(k == KD - 1))
                    nc.vector.tensor_scalar(
                        out=qpt[:, (2 * m) * T:(2 * m) * T + T], in0=qp[0:64, 0:T],
                        scalar1=vv[0:64, VO_QB + m:VO_QB + m + 1], scalar2=None,
                        op0=OP.add)
                    qst = tmp.tile([128, TC], BF16, name="qst", tag="qst", bufs=3)
                    nc.vector.tensor_scalar(
                        out=qst[64:128, 0:T], in0=qp[64:128, 0:T],
                        scalar1=vv[64:128, VO_QB + m:VO_QB + m + 1], scalar2=None,
                        op0=OP.add)
                    nc.scalar.dma_start(qpt[:, (2 * m + 1) * T:(2 * m + 1) * T + T],
                                        qst[64:128, 0:T])

                # ---- KV projection; append K^T and V to caches ----
                kvp = ps.tile([128, 1024], F32, name="kvp", tag="big", bufs=3)
                for k in range(KD):
                    nc.tensor.matmul(kvp[:, 0:TC], kvw_sb[:, k, :], hnT[:, k, :],
                                     start=(k == 0), stop=(k == KD - 1))
                nc.vector.tensor_scalar(
                    out=KC[:, l * T:l * T + T], in0=kvp[0:64, 0:T],
                    scalar1=vv[0:64, VO_KVB:VO_KVB + 1], scalar2=None, op0=OP.add)
                vsb = tmp.tile([128, TC], F32, name="vsb", tag="vsb", bufs=1)
                nc.vector.tensor_scalar(
                    out=vsb[64:128, :], in0=kvp[64:128, 0:TC],
                    scalar1=vv[64:128, VO_KVB:VO_KVB + 1], scalar2=None, op0=OP.add)
                for tc_i, tsz in ((0, 128), (1, 69)):
                    vtp = ps.tile([128, 512], F32, name="vtp", tag="row", bufs=2)
                    nc.tensor.matmul(vtp[0:tsz, 0:HD],
                                     vsb[64:128, tc_i * 128:tc_i * 128 + tsz],
                                     ident[64:128, 64:64 + HD], is_transpose=True,
                                     start=True, stop=True)
                    vts = tmp.tile([128, HD], BF16, name="vts", tag="vts", bufs=2)
                    nc.vector.tensor_copy(vts[0:tsz], vtp[0:tsz, 0:HD])
                    t0 = 0
                    while t0 < tsz:
                        kpos = l * T + tc_i * 128 + t0
                        blk, off = kpos // 128, kpos % 128
                        cnt = min(tsz - t0, 128 - off)
                        nc.scalar.dma_start(
                            VC[off:off + cnt, blk, 0:HD],
                            vts[t0:t0 + cnt, :])
                        t0 += cnt

                # ---- attention ----
                Lk = (l + 1) * T
                nkt = (Lk + 127) // 128
                for qoff, qsz in QP_:
                    nh = (qsz + 511) // 512
                    ot = ps.tile([65, 1024], F32, name="ot", tag="big", bufs=3)
                    for c in range(nkt):
                        ksz = min(128, Lk - c * 128)
                        st = ps.tile([128, 1024], F32, name="st", tag="big", bufs=3)
                        for h in range(nh):
                            cw = min(512, qsz - h * 512)
                            nc.tensor.matmul(
                                st[0:ksz, h * 512:h * 512 + cw],
                                KC[:, c * 128:c * 128 + ksz],
                                qpt[:, qoff + h * 512:qoff + h * 512 + cw],
                                start=True, stop=True)
                        pt = tmp.tile([128, 1024], BF16, name="pt", tag="pt", bufs=2)
                        nc.scalar.activation(pt[0:ksz, 0:qsz], st[0:ksz, 0:qsz],
                                             AF.Exp, scale=SCALE)
                        for h in range(nh):
                            cw = min(512, qsz - h * 512)
                            nc.tensor.matmul(
                                ot[:, h * 512:h * 512 + cw],
                                VC[0:ksz, c, 0:HD + 1],
                                pt[0:ksz, h * 512:h * 512 + cw],
                                start=(c == 0), stop=(c == nkt - 1))
                    # normalize: otn = ot[0:64] * (1/den) broadcast
                    for h in range(nh):
                        cw = min(512, qsz - h * 512)
                        rec = tmp.tile([1, 512], F32R, name="rec", tag="rec", bufs=2)
                        with nc.allow_low_precision(reason="softmax denom f32r"):
                            nc.vector.reciprocal(rec[:, 0:cw],
                                                 ot[64:65, h * 512:h * 512 + cw])
                        rbp = ps.tile([64, 512], F32, name="rbp", tag="row", bufs=2)
                        nc.tensor.matmul(rbp[:, 0:cw], oner[:, 0:64], rec[:, 0:cw],
                                         start=True, stop=True)
                        rb = tmp.tile([64, 512], F32, name="rb", tag="rb", bufs=2)
                        nc.vector.tensor_copy(rb[:, 0:cw], rbp[:, 0:cw])
                        nc.vector.tensor_tensor(
                            otn[:, qoff + h * 512:qoff + h * 512 + cw],
                            ot[0:64, h * 512:h * 512 + cw], rb[:, 0:cw],
                            op=OP.mult)

                # ---- reshape O'T (g,t) -> oT [d, t] ----
                for g in range(G):
                    j, half = g // 2, g % 2
                    if half == 0:
                        nc.vector.tensor_copy(oT[0:64, j, 0:T], otn[:, g * T:g * T + T])
                    else:
                        nc.scalar.dma_start(oT[64:128, j, 0:T], otn[:, g * T:g * T + T])

                # ---- output projection + residual ----
                for m in range(KD):
                    op_ = ps.tile([128, 1024], F32, name="prp", tag="big", bufs=3)
                    for k in range(KD):
                        nc.tensor.matmul(op_[:, 0:TC], ow_sb[:, k, m * 128:(m + 1) * 128],
                                         oT[:, k, :], start=(k == 0), stop=(k == KD - 1))
                    nc.vector.scalar_tensor_tensor(
                        out=hT[:, m, 0:T], in0=op_[:, 0:T],
                        scalar=vv[:, VO_PB + m:VO_PB + m + 1],
                        in1=hT[:, m, 0:T], op0=OP.add, op1=OP.add)

                # ---- MLP ----
                ln(hnT, vv, VO_L2W, VO_L2B)
                for m in range(MD):
                    f1s = f1q[m // 6]
                    mi = m % 6
                    fp = ps.tile([128, 1024], F32, name="fp", tag="big", bufs=3)
                    for k in range(KD):
                        nc.tensor.matmul(fp[:, 0:TC], f1s[:, k, mi * 128:(mi + 1) * 128],
                                         hnT[:, k, :], start=(k == 0), stop=(k == KD - 1))
                    nc.scalar.activation(g1T[:, m, :], fp[:, 0:TC], AF.Gelu,
                                         bias=vv[:, VO_F1B + m:VO_F1B + m + 1])
                for m in range(KD):
                    f2p = ps.tile([128, 1024], F32, name="f2p", tag="big", bufs=3)
                    for k in range(MD):
                        f2s = f2q[k // 6]
                        nc.tensor.matmul(f2p[:, 0:TC],
                                         f2s[:, k % 6, m * 128:(m + 1) * 128],
                                         g1T[:, k, :], start=(k == 0), stop=(k == MD - 1))
                    nc.vector.scalar_tensor_tensor(
                        out=hT[:, m, 0:T], in0=f2p[:, 0:T],
                        scalar=vv[:, VO_F2B + m:VO_F2B + m + 1],
                        in1=hT[:, m, 0:T], op0=OP.add, op1=OP.add)

                if l + 1 < DEPTH:
                    lw = nxt

            # ---------------- final LN + head ----------------
            nv = persist.tile([128, 12], F32)
            nc.sync.dma_start(nv, d["normv"].ap())
            nc.sync.dma_start(orow, d["headb"].ap().rearrange("(o c) -> o c", o=1))
            ln(hnT, nv, 0, 6)
            hwr = d["headw"].ap().rearrange("p (k c) -> p k c", k=KD)
            for n in range(2):
                hw_c = wsl.tile([128, KD, 500], BF16, name="hw_c", tag="slab")
                nc.sync.dma_start(hw_c, hwr[:, :, n * 500:(n + 1) * 500])
                hp = ps.tile([1, 512], F32, name="hp", tag="row", bufs=2)
                for k in range(KD):
                    nc.tensor.matmul(hp[:, 0:500], hnT[:, k, 0:1], hw_c[:, k, :],
                                     start=(k == 0), stop=(k == KD - 1))
                nc.vector.tensor_tensor(orow[:, n * 500:(n + 1) * 500], hp[:, 0:500],
                                        orow[:, n * 500:(n + 1) * 500], op=OP.add)
            nc.sync.dma_start(out_d.ap(), orow)

    nc.compile()
    return nc


def _tile_w(w):
    """(K*128, C) fp32 -> (128, K*C) bf16 tiled: out[p, k*C+c] = w[k*128+p, c]."""
    k = w.shape[0] // 128
    c = w.shape[1]
    return np.ascontiguousarray(
        w.reshape(k, 128, c).transpose(1, 0, 2).reshape(128, k * c).astype(BFNP))


def _vcol(v):
    """(K*128,) -> (128, K): out[p, k] = v[k*128+p]."""
    k = v.shape[0] // 128
    return v.reshape(k, 128).T


def make_in_maps(inputs):
    f = {n: np.asarray(inputs[n], dtype=np.float32) for n in inputs}

    shared = {}
    shared["patchw"] = _tile_w(f["patch_w"])
    shared["qw"] = np.stack([_tile_w(f["q_w"][l]) for l in range(DEPTH)])
    shared["kvw"] = np.stack([_tile_w(f["kv_w"][l]) for l in range(DEPTH)])
    shared["projw"] = np.stack([_tile_w(f["proj_w"][l]) for l in range(DEPTH)])
    shared["fc1w"] = np.stack([_tile_w(f["fc1_w"][l]) for l in range(DEPTH)])
    shared["fc2w"] = np.stack([_tile_w(f["fc2_w"][l]) for l in range(DEPTH)])
    shared["headw"] = _tile_w(f["head_w"])
    shared["headb"] = f["head_b"]

    vecs = np.zeros((DEPTH, 128, NV), np.float32)
    for l in range(DEPTH):
        vecs[l, :, VO_L1W:VO_L1W + 6] = _vcol(f["ln1_w"][l])
        vecs[l, :, VO_L1B:VO_L1B + 6] = _vcol(f["ln1_b"][l])
        vecs[l, :, VO_QB:VO_QB + 6] = _vcol(f["q_b"][l])
        vecs[l, :, VO_KVB] = f["kv_b"][l]
        vecs[l, :, VO_PB:VO_PB + 6] = _vcol(f["proj_b"][l])
        vecs[l, :, VO_L2W:VO_L2W + 6] = _vcol(f["ln2_w"][l])
        vecs[l, :, VO_L2B:VO_L2B + 6] = _vcol(f["ln2_b"][l])
        vecs[l, :, VO_F1B:VO_F1B + 24] = _vcol(f["fc1_b"][l])
        vecs[l, :, VO_F2B:VO_F2B + 6] = _vcol(f["fc2_b"][l])
    shared["vecs"] = np.ascontiguousarray(vecs)

    normv = np.zeros((128, 12), np.float32)
    normv[:, 0:6] = _vcol(f["norm_w"])
    normv[:, 6:12] = _vcol(f["norm_b"])
    shared["normv"] = normv

    # pos_embed + patch_b / cls folding, transposed token layout
    posb = np.zeros((DIM, TC), np.float32)
    posb[:, 0] = f["cls_token"][0, 0] + f["pos_embed"][0, 0]
    posb[:, 1:T] = (f["pos_embed"][0, 1:T] + f["patch_b"][None, :]).T
    shared["posbt"] = np.ascontiguousarray(
        posb.reshape(KD, 128, TC).transpose(1, 0, 2).reshape(128, KD * TC))

    shared["_ones"] = np.ones((128,), np.float32)
    shared["_zeros"] = np.zeros((KD * TR,), np.float32)

    # per-core im2col (transposed): xpt[(c,a,b), 1 + i*14 + j]
    HG = IMG // P
    x = np.asarray(inputs["x"], dtype=np.float32)
    maps = []
    for b in range(B):
        xp = x[b].reshape(CIN, HG, P, HG, P).transpose(0, 2, 4, 1, 3)
        xp = xp.reshape(DIM, NPATCH)
        xt = np.zeros((DIM, TC), np.float32)
        xt[:, 1:T] = xp
        xt = xt.reshape(KD, 128, TC).transpose(1, 0, 2).reshape(128, KD * TC)
        maps.append(dict(shared, xpt=np.ascontiguousarray(xt.astype(BFNP))))
    return maps


def kernel(**inputs):
    if "nc" not in _CACHED:
        _CACHED["nc"] = build_module()
    nc = _CACHED["nc"]
    res = run_bass_kernel_spmd(nc, make_in_maps(inputs), core_ids=list(range(B)))
    return np.concatenate([res.results[b]["out"] for b in range(B)], axis=0)


# revision 22
# speedup vs baseline: 1.7646x; 1.0974x over previous
"""MoDA Vision Transformer forward pass on 8 Trainium2 NeuronCores.

Sharding: pure data-parallel over batch (B=8 -> 1 image per core, weights
replicated, no collectives).

v2 design (bf16 compute, fp32 residual):
- All weights pre-tiled + cast to bf16 on HOST into exactly the SBUF layout
  [p, k, c], so every weight DMA is one contiguous chunk per partition
  (128 descriptors per load instead of ~10k strided ones).
- Residual stream hT stays fp32 (f32r) padded to 256 token cols so the
  LayerNorm sum matmuls run at full f32r rate; all other activations are
  bf16 at 208 token cols (bf16 matmuls are full rate at any width).
- im2col + pos_embed/cls/bias folding done on host; patch embed is a
  plain linear directly into the transposed residual layout.
- Attention (GQA kv_heads=1, depth-KV cache): q pieces of (1024,1024,316)
  flattened (g,t) queries; per kpos-block one 1024-wide exp on ACT
  (amortizes the 352-cycle ACTIVATE overhead); denominators via a ones
  column appended to V; softmax normalize with DVE reciprocal.
- PSUM: 3x [128,1024] "big" slots (6 banks) + 2x [1-128,512] "row" slots
  (2 banks) = exactly 8 banks.
- Next layer's weights are prefetched at the top of each layer body on the
  sync HWDGE ring; small strip DMAs ride the scalar HWDGE ring.
"""

import numpy as np
import ml_dtypes

import concourse.bass as bass
import concourse.mybir as mybir
import concourse.tile as tile
from concourse import bacc
from concourse.bass_utils import run_bass_kernel_spmd
from concourse.masks import make_identity

F32 = mybir.dt.float32
F32R = mybir.dt.float32r
BF16 = mybir.dt.bfloat16
AF = mybir.ActivationFunctionType
OP = mybir.AluOpType
BFNP = ml_dtypes.bfloat16

B, CIN, IMG, P = 8, 3, 224, 16
DIM, DEPTH, NH, NKV = 768, 12, 12, 1
HD = DIM // NH              # 64
G = NH // NKV               # 12
NPATCH = (IMG // P) ** 2    # 196
T = NPATCH + 1              # 197
TC = 208                    # bf16 activation token cols (197 padded)
TR = 256                    # fp32 residual token cols (f32r full-rate >=256)
KD = DIM // 128             # 6
MLP = 4 * DIM               # 3072
MD = MLP // 128             # 24
NQ = G * T                  # 2364
NQP = 2368                  # padded
NKBLK = (DEPTH * T + 127) // 128  # 19
VW = 130                    # V block width (64 V + 1 ones + 63 zero + pad)
SCALE = HD ** -0.5
EPS = 1e-6
NCLS = 1000
QP_ = [(0, 1024), (1024, 1024), (2048, NQ - 2048)]  # q pieces

# packed per-layer vector columns (fp32): [128, NV]
VO_L1W, VO_L1B, VO_QB, VO_KVB, VO_PB = 0, 6, 12, 18, 19
VO_L2W, VO_L2B, VO_F1B, VO_F2B, NV = 25, 31, 37, 61, 67

_CACHED = {}


def build_module():
    nc = bacc.Bacc("TRN2", target_bir_lowering=False, debug=False, num_devices=8)

    d = {}
    def din(name, shape, dt_):
        d[name] = nc.dram_tensor(name, shape, dt_, kind="ExternalInput")

    din("xpt", (128, KD * TC), BF16)
    din("posbt", (128, KD * TC), F32)
    din("patchw", (128, KD * DIM), BF16)
    din("qw", (DEPTH, 128, KD * DIM), BF16)
    din("kvw", (DEPTH, 128, KD * 2 * HD), BF16)
    din("projw", (DEPTH, 128, KD * DIM), BF16)
    din("fc1w", (DEPTH, 128, KD * MLP), BF16)
    din("fc2w", (DEPTH, 128, MD * DIM), BF16)
    din("vecs", (DEPTH, 128, NV), F32)
    din("normv", (128, 12), F32)
    din("headw", (128, KD * NCLS), BF16)
    din("headb", (NCLS,), F32)
    din("_ones", (128,), F32R)
    din("_zeros", (KD * TR,), F32R)
    out_d = nc.dram_tensor("out", (1, NCLS), F32, kind="ExternalOutput")

    with tile.TileContext(nc) as tc:
        with (
            tc.tile_pool(name="persist", bufs=1) as persist,
            tc.tile_pool(name="wq", bufs=2) as wq,        # q/kv/proj + vecs
            tc.tile_pool(name="wsl", bufs=10) as wsl,     # fc1/fc2 quarter slabs
            tc.tile_pool(name="tmp", bufs=2) as tmp,      # transient sbuf
            tc.tile_pool(name="ps", bufs=1, space="PSUM") as ps,
        ):
            # ---------------- persistent state ----------------
            hT = persist.tile([128, KD, TR], F32R)          # residual ^T (fp32)
            sqT = persist.tile([128, KD, TR], BF16)         # squares scratch
            hnT = persist.tile([128, KD, TC], BF16)         # LN output ^T
            oT = persist.tile([128, KD, TC], BF16)          # attn out ^T
            g1T = persist.tile([128, MD, TC], BF16)         # gelu(fc1) ^T
            KC = persist.tile([128, NQP], BF16)             # K cache ^T (x2 dup)
            VC = persist.tile([128, NKBLK, VW], BF16)       # V cache + ones col
            qpt = persist.tile([128, NQP], BF16)            # Q'^T (g,t) (x2 dup)
            otn = persist.tile([64, NQP], BF16)             # normalized O'^T
            ident = persist.tile([128, 128], F32)
            onec = persist.tile([128, 1], F32R)             # 1.0 col (LN sum lhsT)
            onecb = persist.tile([128, 1], BF16)
            oner = persist.tile([1, 128], F32R)             # 1.0 row (bcast lhsT)
            eps_t = persist.tile([1, 1], F32)
            orow = persist.tile([1, NCLS], F32)

            nc.gpsimd.dma_start(
                hT, d["_zeros"].ap().rearrange("(o c) -> o c", o=1)
                .to_broadcast([128, KD * TR]))
            nc.vector.memset(oT, 0.0)
            nc.vector.memset(VC, 0.0)
            make_identity(nc, ident)
            nc.sync.dma_start(onec, d["_ones"].ap().rearrange("(p o) -> p o", o=1))
            nc.sync.dma_start(oner, d["_ones"].ap().rearrange("(o p) -> o p", o=1))
            nc.vector.memset(onecb, 1.0)
            nc.vector.memset(eps_t, EPS)
            nc.vector.memset(VC[:, :, HD:HD + 2], 1.0)

            # ---------------- weight loading helpers ----------------
            def load_qkvp(l):
                v = wq.tile([128, NV], F32, name="vecs", tag="vecs")
                nc.sync.dma_start(v, d["vecs"].ap()[l])
                qw_ = wq.tile([128, KD, DIM], BF16, name="qw", tag="qw")
                nc.sync.dma_start(qw_, d["qw"].ap()[l].rearrange("p (k c) -> p k c", k=KD))
                kvw_ = wq.tile([128, KD, 2 * HD], BF16, name="kvw", tag="kvw")
                nc.sync.dma_start(kvw_, d["kvw"].ap()[l].rearrange("p (k c) -> p k c", k=KD))
                ow_ = wq.tile([128, KD, DIM], BF16, name="ow", tag="ow")
                nc.sync.dma_start(ow_, d["projw"].ap()[l].rearrange("p (k c) -> p k c", k=KD))
                return v, qw_, kvw_, ow_

            def load_slabs(l):
                f1r = d["fc1w"].ap()[l].rearrange("p (k c) -> p k c", k=KD)
                f2r = d["fc2w"].ap()[l].rearrange("p (k c) -> p k c", k=MD)
                f1q, f2q = [], []
                for i in range(4):  # fc1 quarter: m-tiles 6i/4.. (768 cols each)
                    s = wsl.tile([128, KD, MLP // 4], BF16, name="f1q", tag="slab")
                    nc.sync.dma_start(s, f1r[:, :, i * (MLP // 4):(i + 1) * (MLP // 4)])
                    f1q.append(s)
                for i in range(4):  # fc2 quarter: k-tiles 6i..6i+5 (full 768 cols)
                    s = wsl.tile([128, KD, DIM], BF16, name="f2q", tag="slab")
                    nc.sync.dma_start(s, f2r[:, i * KD:(i + 1) * KD, :])
                    f2q.append(s)
                return f1q, f2q

            # ---------------- layernorm ----------------
            def ln(dst, wb, wo, bo):
                """LN over d of hT -> dst[128, KD, TC] (bf16).

                wb: [128, NV]-style tile; wo/bo: col offsets of gamma/beta."""
                nc.vector.tensor_tensor(sqT, hT, hT, op=OP.mult)
                ssum = ps.tile([1, 512], F32, name="ssum", tag="row", bufs=2)
                ssq = ps.tile([1, 512], F32, name="ssq", tag="row", bufs=2)
                for k in range(KD):
                    nc.tensor.matmul(ssum[:, 0:TR], onec, hT[:, k, :],
                                     start=(k == 0), stop=(k == KD - 1))
                for k in range(KD):
                    nc.tensor.matmul(ssq[:, 0:TR], onecb, sqT[:, k, :],
                                     start=(k == 0), stop=(k == KD - 1))
                mean = tmp.tile([1, TC], F32, name="mean", tag="mean", bufs=1)
                nc.vector.tensor_scalar(out=mean, in0=ssum[:, 0:TC],
                                        scalar1=1.0 / DIM, scalar2=None, op0=OP.mult)
                m2 = tmp.tile([1, TC], F32, name="m2", tag="m2", bufs=1)
                nc.vector.tensor_tensor(m2, mean, mean, op=OP.mult)
                var = tmp.tile([1, TC], F32, name="var", tag="var", bufs=1)
                nc.vector.scalar_tensor_tensor(
                    out=var, in0=ssq[:, 0:TC], scalar=1.0 / DIM,
                    in1=m2, op0=OP.mult, op1=OP.subtract)
                lnv = tmp.tile([1, TC], F32, name="lnv", tag="lnv", bufs=1)
                nc.scalar.activation(lnv, var, AF.Ln, bias=eps_t)
                rstd = tmp.tile([1, TC], F32R, name="rstd", tag="rstd", bufs=1)
                nc.scalar.activation(rstd, lnv, AF.Exp, scale=-0.5)
                mr = tmp.tile([1, TC], F32R, name="mr", tag="mr", bufs=1)
                nc.vector.tensor_tensor(mr, mean, rstd, op=OP.mult)
                rstd_b = ps.tile([128, TC], F32, name="rstd_b", tag="row", bufs=2)
                mr_b = ps.tile([128, TC], F32, name="mr_b", tag="row", bufs=2)
                nc.tensor.matmul(rstd_b, oner, rstd, start=True, stop=True)
                nc.tensor.matmul(mr_b, oner, mr, start=True, stop=True)
                for k in range(KD):
                    t1 = tmp.tile([128, TC], F32, name="lnt", tag="lnt", bufs=2)
                    nc.vector.tensor_tensor(t1, hT[:, k, 0:TC], rstd_b, op=OP.mult)
                    nc.vector.tensor_tensor(t1, t1, mr_b, op=OP.subtract)
                    nc.vector.tensor_scalar(
                        out=dst[:, k, :], in0=t1,
                        scalar1=wb[:, wo + k:wo + k + 1],
                        scalar2=wb[:, bo + k:bo + k + 1],
                        op0=OP.mult, op1=OP.add)

            # ---------------- prologue: patch embed ----------------
            xpt = tmp.tile([128, KD, TC], BF16, name="xpt", tag="pt", bufs=2)
            nc.sync.dma_start(xpt, d["xpt"].ap().rearrange("p (k c) -> p k c", k=KD))
            posbt = tmp.tile([128, KD, TC], F32, name="posbt", tag="posbt", bufs=1)
            nc.sync.dma_start(posbt, d["posbt"].ap().rearrange("p (k c) -> p k c", k=KD))
            pw_sb = wq.tile([128, KD, DIM], BF16, name="pw_sb", tag="qw")
            nc.sync.dma_start(pw_sb, d["patchw"].ap().rearrange("p (k c) -> p k c", k=KD))
            vecs0 = load_qkvp(0)
            slabs0 = load_slabs(0)

            for m in range(KD):
                pp = ps.tile([128, 1024], F32, name="pp", tag="big", bufs=3)
                for k in range(KD):
                    nc.tensor.matmul(pp[:, 0:TC], pw_sb[:, k, m * 128:(m + 1) * 128],
                                     xpt[:, k, :], start=(k == 0), stop=(k == KD - 1))
                nc.vector.tensor_tensor(hT[:, m, 0:TC], pp[:, 0:TC],
                                        posbt[:, m, :], op=OP.add)

            # ---------------- transformer layers ----------------
            lw = (vecs0, slabs0)
            for l in range(DEPTH):
                (vv, qw_sb, kvw_sb, ow_sb), (f1q, f2q) = lw
                if l + 1 < DEPTH:
                    nxt = (load_qkvp(l + 1), load_slabs(l + 1))

                ln(hnT, vv, VO_L1W, VO_L1B)

                # ---- Q projection -> qpt strips ----
                for m in range(KD):
                    qp = ps.tile([128, 1024], F32, name="qp", tag="big", bufs=3)
                    for k in range(KD):
                        nc.tensor.matmul(qp[:, 0:TC], qw_sb[:, k, m * 128:(m + 1) * 128],
                                         hnT[:, k, :], start=(k == 0), stop=(k == KD - 1))
                    nc.vector.tensor_scalar(
                        out=qpt[0:64, (2 * m) * T:(2 * m) * T + T], in0=qp[0:64, 0:T],
                        scalar1=vv[0:64, VO_QB + m:VO_QB + m + 1], scalar2=None,
                        op0=OP.add)
                    qst = tmp.tile([128, TC], BF16, name="qst", tag="qst", bufs=3)
                    nc.vector.tensor_scalar(
                        out=qst[64:128, 0:T], in0=qp[64:128, 0:T],
                        scalar1=vv[64:128, VO_QB + m:VO_QB + m + 1], scalar2=None,
                        op0=OP.add)
                    nc.scalar.dma_start(
                        qpt[0:64, (2 * m + 1) * T:(2 * m + 1) * T + T],
                        qst[64:128, 0:T])

                # ---- KV projection; append K^T and V to caches ----
                kvp = ps.tile([128, 1024], F32, name="kvp", tag="big", bufs=3)
                for k in range(KD):
                    nc.tensor.matmul(kvp[:, 0:TC], kvw_sb[:, k, :], hnT[:, k, :],
                                     start=(k == 0), stop=(k == KD - 1))
                nc.vector.tensor_scalar(
                    out=KC[0:64, l * T:l * T + T], in0=kvp[0:64, 0:T],
                    scalar1=vv[0:64, VO_KVB:VO_KVB + 1], scalar2=None, op0=OP.add)
                nc.scalar.dma_start(KC[64:128, l * T:l * T + T],
                                    KC[0:64, l * T:l * T + T])
                vsb = tmp.tile([128, TC], F32, name="vsb", tag="vsb", bufs=1)
                nc.vector.tensor_scalar(
                    out=vsb[64:128, :], in0=kvp[64:128, 0:TC],
                    scalar1=vv[64:128, VO_KVB:VO_KVB + 1], scalar2=None, op0=OP.add)
                for tc_i, tsz in ((0, 128), (1, 69)):
                    vtp = ps.tile([128, 512], F32, name="vtp", tag="row", bufs=2)
                    nc.tensor.matmul(vtp[0:tsz, 0:HD],
                                     vsb[64:128, tc_i * 128:tc_i * 128 + tsz],
                                     ident[64:128, 64:64 + HD], is_transpose=True,
                                     start=True, stop=True)
                    vts = tmp.tile([128, HD], BF16, name="vts", tag="vts", bufs=2)
                    nc.vector.tensor_copy(vts[0:tsz], vtp[0:tsz, 0:HD])
                    t0 = 0
                    while t0 < tsz:
                        kpos = l * T + tc_i * 128 + t0
                        blk, off = kpos // 128, kpos % 128
                        cnt = min(tsz - t0, 128 - off)
                        nc.scalar.dma_start(
                            VC[off:off + cnt, blk, 0:HD],
                            vts[t0:t0 + cnt, :])
                        t0 += cnt

                nc.scalar.dma_start(qpt[64:128, 0:NQ], qpt[0:64, 0:NQ])

                # ---- attention ----
                Lk = (l + 1) * T
                nkt = (Lk + 127) // 128
                for qoff, qsz in QP_:
                    nh = (qsz + 511) // 512
                    ot = ps.tile([128, 1024], F32, name="ot", tag="big", bufs=3)
                    for c in range(nkt):
                        ksz = min(128, Lk - c * 128)
                        st = ps.tile([128, 1024], F32, name="st", tag="big", bufs=3)
                        for h in range(nh):
                            cw = min(512, qsz - h * 512)
                            nc.tensor.matmul(
                                st[0:ksz, h * 512:h * 512 + cw],
                                KC[:, c * 128:c * 128 + ksz],
                                qpt[:, qoff + h * 512:qoff + h * 512 + cw],
                                start=True, stop=True)
                        pt = tmp.tile([128, 1024], BF16, name="pt", tag="pt", bufs=2)
                        nc.scalar.activation(pt[0:ksz, 0:qsz], st[0:ksz, 0:qsz],
                                             AF.Exp, scale=SCALE / 2)
                        for h in range(nh):
                            cw = min(512, qsz - h * 512)
                            nc.tensor.matmul(
                                ot[:, h * 512:h * 512 + cw],
                                VC[0:ksz, c, 0:128],
                                pt[0:ksz, h * 512:h * 512 + cw],
                                start=(c == 0), stop=(c == nkt - 1))
                    # normalize: otn = ot[0:64] * (1/den) broadcast
                    for h in range(nh):
                        cw = min(512, qsz - h * 512)
                        denr = tmp.tile([1, 512], F32R, name="denr", tag="rec", bufs=2)
                        nc.vector.tensor_copy(denr[:, 0:cw],
                                              ot[64:65, h * 512:h * 512 + cw])
                        rbp = ps.tile([64, 512], F32, name="rbp", tag="row", bufs=2)
                        nc.tensor.matmul(rbp[:, 0:cw], oner[:, 0:64], denr[:, 0:cw],
                                         start=True, stop=True)
                        rb = tmp.tile([64, 512], F32, name="rb", tag="rb", bufs=2)
                        nc.vector.reciprocal_approx_fast(rb[:, 0:cw], rbp[:, 0:cw])
                        nc.vector.tensor_tensor(
                            otn[:, qoff + h * 512:qoff + h * 512 + cw],
                            ot[0:64, h * 512:h * 512 + cw], rb[:, 0:cw],
                            op=OP.mult)

                # ---- reshape O'T (g,t) -> oT [d, t] ----
                for g in range(G):
                    j, half = g // 2, g % 2
                    if half == 0:
                        nc.vector.tensor_copy(oT[0:64, j, 0:T], otn[:, g * T:g * T + T])
                    else:
                        nc.scalar.dma_start(oT[64:128, j, 0:T], otn[:, g * T:g * T + T])

                # ---- output projection + residual ----
                for m in range(KD):
                    op_ = ps.tile([128, 1024], F32, name="prp", tag="big", bufs=3)
                    for k in range(KD):
                        nc.tensor.matmul(op_[:, 0:TC], ow_sb[:, k, m * 128:(m + 1) * 128],
                                         oT[:, k, :], start=(k == 0), stop=(k == KD - 1))
                    nc.vector.scalar_tensor_tensor(
                        out=hT[:, m, 0:T], in0=op_[:, 0:T],
                        scalar=vv[:, VO_PB + m:VO_PB + m + 1],
                        in1=hT[:, m, 0:T], op0=OP.add, op1=OP.add)

                # ---- MLP ----
                ln(hnT, vv, VO_L2W, VO_L2B)
                for m in range(MD):
                    f1s = f1q[m // 6]
                    mi = m % 6
                    fp = ps.tile([128, 1024], F32, name="fp", tag="big", bufs=3)
                    for k in range(KD):
                        nc.tensor.matmul(fp[:, 0:TC], f1s[:, k, mi * 128:(mi + 1) * 128],
                                         hnT[:, k, :], start=(k == 0), stop=(k == KD - 1))
                    nc.scalar.activation(g1T[:, m, :], fp[:, 0:TC], AF.Gelu,
                                         bias=vv[:, VO_F1B + m:VO_F1B + m + 1])
                for m in range(KD):
                    f2p = ps.tile([128, 1024], F32, name="f2p", tag="big", bufs=3)
                    for k in range(MD):
                        f2s = f2q[k // 6]
                        nc.tensor.matmul(f2p[:, 0:TC],
                                         f2s[:, k % 6, m * 128:(m + 1) * 128],
                                         g1T[:, k, :], start=(k == 0), stop=(k == MD - 1))
                    nc.vector.scalar_tensor_tensor(
                        out=hT[:, m, 0:T], in0=f2p[:, 0:T],
                        scalar=vv[:, VO_F2B + m:VO_F2B + m + 1],
                        in1=hT[:, m, 0:T], op0=OP.add, op1=OP.add)

                if l + 1 < DEPTH:
                    lw = nxt

            # ---------------- final LN + head ----------------
            nv = persist.tile([128, 12], F32)
            nc.sync.dma_start(nv, d["normv"].ap())
            nc.sync.dma_start(orow, d["headb"].ap().rearrange("(o c) -> o c", o=1))
            ln(hnT, nv, 0, 6)
            hwr = d["headw"].ap().rearrange("p (k c) -> p k c", k=KD)
            for n in range(2):
                hw_c = wsl.tile([128, KD, 500], BF16, name="hw_c", tag="slab")
                nc.sync.dma_start(hw_c, hwr[:, :, n * 500:(n + 1) * 500])
                hp = ps.tile([1, 512], F32, name="hp", tag="row", bufs=2)
                for k in range(KD):
                    nc.tensor.matmul(hp[:, 0:500], hnT[:, k, 0:1], hw_c[:, k, :],
                                     start=(k == 0), stop=(k == KD - 1))
                nc.vector.tensor_tensor(orow[:, n * 500:(n + 1) * 500], hp[:, 0:500],
                                        orow[:, n * 500:(n + 1) * 500], op=OP.add)
            nc.sync.dma_start(out_d.ap(), orow)

    nc.compile()
    return nc


def _tile_w(w):
    """(K*128, C) fp32 -> (128, K*C) bf16 tiled: out[p, k*C+c] = w[k*128+p, c]."""
    k = w.shape[0] // 128
    c = w.shape[1]
    return np.ascontiguousarray(
        w.reshape(k, 128, c).transpose(1, 0, 2).reshape(128, k * c).astype(BFNP))


def _vcol(v):
    """(K*128,) -> (128, K): out[p, k] = v[k*128+p]."""
    k = v.shape[0] // 128
    return v.reshape(k, 128).T


def make_in_maps(inputs):
    f = {n: np.asarray(inputs[n], dtype=np.float32) for n in inputs}

    shared = {}
    shared["patchw"] = _tile_w(f["patch_w"])
    shared["qw"] = np.stack([_tile_w(f["q_w"][l]) for l in range(DEPTH)])
    shared["kvw"] = np.stack([_tile_w(f["kv_w"][l]) for l in range(DEPTH)])
    shared["projw"] = np.stack([_tile_w(f["proj_w"][l]) for l in range(DEPTH)])
    shared["fc1w"] = np.stack([_tile_w(f["fc1_w"][l]) for l in range(DEPTH)])
    shared["fc2w"] = np.stack([_tile_w(f["fc2_w"][l]) for l in range(DEPTH)])
    shared["headw"] = _tile_w(f["head_w"])
    shared["headb"] = f["head_b"]

    vecs = np.zeros((DEPTH, 128, NV), np.float32)
    for l in range(DEPTH):
        vecs[l, :, VO_L1W:VO_L1W + 6] = _vcol(f["ln1_w"][l])
        vecs[l, :, VO_L1B:VO_L1B + 6] = _vcol(f["ln1_b"][l])
        vecs[l, :, VO_QB:VO_QB + 6] = _vcol(f["q_b"][l])
        vecs[l, :, VO_KVB] = f["kv_b"][l]
        vecs[l, :, VO_PB:VO_PB + 6] = _vcol(f["proj_b"][l])
        vecs[l, :, VO_L2W:VO_L2W + 6] = _vcol(f["ln2_w"][l])
        vecs[l, :, VO_L2B:VO_L2B + 6] = _vcol(f["ln2_b"][l])
        vecs[l, :, VO_F1B:VO_F1B + 24] = _vcol(f["fc1_b"][l])
        vecs[l, :, VO_F2B:VO_F2B + 6] = _vcol(f["fc2_b"][l])
    shared["vecs"] = np.ascontiguousarray(vecs)

    normv = np.zeros((128, 12), np.float32)
    normv[:, 0:6] = _vcol(f["norm_w"])
    normv[:, 6:12] = _vcol(f["norm_b"])
    shared["normv"] = normv

    # pos_embed + patch_b / cls folding, transposed token layout
    posb = np.zeros((DIM, TC), np.float32)
    posb[:, 0] = f["cls_token"][0, 0] + f["pos_embed"][0, 0]
    posb[:, 1:T] = (f["pos_embed"][0, 1:T] + f["patch_b"][None, :]).T
    shared["posbt"] = np.ascontiguousarray(
        posb.reshape(KD, 128, TC).transpose(1, 0, 2).reshape(128, KD * TC))

    shared["_ones"] = np.ones((128,), np.float32)
    shared["_zeros"] = np.zeros((KD * TR,), np.float32)

    # per-core im2col (transposed): xpt[(c,a,b), 1 + i*14 + j]
    HG = IMG // P
    x = np.asarray(inputs["x"], dtype=np.float32)
    maps = []
    for b in range(B):
        xp = x[b].reshape(CIN, HG, P, HG, P).transpose(0, 2, 4, 1, 3)
        xp = xp.reshape(DIM, NPATCH)
        xt = np.zeros((DIM, TC), np.float32)
        xt[:, 1:T] = xp
        xt = xt.reshape(KD, 128, TC).transpose(1, 0, 2).reshape(128, KD * TC)
        maps.append(dict(shared, xpt=np.ascontiguousarray(xt.astype(BFNP))))
    return maps


def kernel(**inputs):
    if "nc" not in _CACHED:
        _CACHED["nc"] = build_module()
    nc = _CACHED["nc"]
    res = run_bass_kernel_spmd(nc, make_in_maps(inputs), core_ids=list(range(B)))
    return np.concatenate([res.results[b]["out"] for b in range(B)], axis=0)


# revision 23
# speedup vs baseline: 1.8760x; 1.0632x over previous
"""MoDA Vision Transformer forward pass on 8 Trainium2 NeuronCores.

Sharding: pure data-parallel over batch (B=8 -> 1 image per core, weights
replicated, no collectives).

v2 design (bf16 compute, fp32 residual):
- All weights pre-tiled + cast to bf16 on HOST into exactly the SBUF layout
  [p, k, c], so every weight DMA is one contiguous chunk per partition
  (128 descriptors per load instead of ~10k strided ones).
- Residual stream hT stays fp32 (f32r) padded to 256 token cols so the
  LayerNorm sum matmuls run at full f32r rate; all other activations are
  bf16 at 208 token cols (bf16 matmuls are full rate at any width).
- im2col + pos_embed/cls/bias folding done on host; patch embed is a
  plain linear directly into the transposed residual layout.
- Attention (GQA kv_heads=1, depth-KV cache): q pieces of (1024,1024,316)
  flattened (g,t) queries; per kpos-block one 1024-wide exp on ACT
  (amortizes the 352-cycle ACTIVATE overhead); denominators via a ones
  column appended to V; softmax normalize with DVE reciprocal.
- PSUM: 3x [128,1024] "big" slots (6 banks) + 2x [1-128,512] "row" slots
  (2 banks) = exactly 8 banks.
- Next layer's weights are prefetched at the top of each layer body on the
  sync HWDGE ring; small strip DMAs ride the scalar HWDGE ring.
"""

import numpy as np
import ml_dtypes

import concourse.bass as bass
import concourse.mybir as mybir
import concourse.tile as tile
from concourse import bacc
from concourse.bass_utils import run_bass_kernel_spmd
from concourse.masks import make_identity

F32 = mybir.dt.float32
F32R = mybir.dt.float32r
BF16 = mybir.dt.bfloat16
I32 = mybir.dt.int32
AF = mybir.ActivationFunctionType
OP = mybir.AluOpType
BFNP = ml_dtypes.bfloat16

B, CIN, IMG, P = 8, 3, 224, 16
DIM, DEPTH, NH, NKV = 768, 12, 12, 1
HD = DIM // NH              # 64
G = NH // NKV               # 12
NPATCH = (IMG // P) ** 2    # 196
T = NPATCH + 1              # 197
TC = 208                    # bf16 activation token cols (197 padded)
TR = 256                    # fp32 residual token cols (f32r full-rate >=256)
KD = DIM // 128             # 6
MLP = 4 * DIM               # 3072
MD = MLP // 128             # 24
NQ = G * T                  # 2364
NQP = 2368                  # padded
NKBLK = (DEPTH * T + 127) // 128  # 19
VW = 130                    # V block width (64 V + 1 ones + 63 zero + pad)
SCALE = HD ** -0.5
EPS = 1e-6
NCLS = 1000
QP_ = [(0, 1024), (1024, 1024), (2048, NQ - 2048)]  # q pieces

# packed per-layer vector columns (fp32): [128, NV]
VO_L1W, VO_L1B, VO_QB, VO_KVB, VO_PB = 0, 6, 12, 18, 19
VO_L2W, VO_L2B, VO_F1B, VO_F2B, NV = 25, 31, 37, 61, 67

_CACHED = {}


def build_module():
    nc = bacc.Bacc("TRN2", target_bir_lowering=False, debug=False, num_devices=8)

    d = {}
    def din(name, shape, dt_):
        d[name] = nc.dram_tensor(name, shape, dt_, kind="ExternalInput")

    din("xpt", (128, KD * TC), BF16)
    din("posbt", (128, KD * TC), F32)
    din("patchw", (128, KD * DIM), BF16)
    din("qw", (DEPTH, 128, KD * DIM), BF16)
    din("kvw", (DEPTH, 128, KD * 2 * HD), BF16)
    din("projw", (DEPTH, 128, KD * DIM), BF16)
    din("fc1w", (DEPTH, 128, KD * MLP), BF16)
    din("fc2w", (DEPTH, 128, MD * DIM), BF16)
    din("vecs", (DEPTH, 128, NV), F32)
    din("normv", (128, 12), F32)
    din("headw", (128, KD * NCLS), BF16)
    din("headb", (NCLS,), F32)
    din("_ones", (128,), F32R)
    din("_zeros", (KD * TR,), F32R)
    out_d = nc.dram_tensor("out", (1, NCLS), F32, kind="ExternalOutput")

    with tile.TileContext(nc) as tc:
        with (
            tc.tile_pool(name="persist", bufs=1) as persist,
            tc.tile_pool(name="wq", bufs=2) as wq,        # q/kv/proj + vecs
            tc.tile_pool(name="wsl", bufs=10) as wsl,     # fc1/fc2 quarter slabs
            tc.tile_pool(name="tmp", bufs=2) as tmp,      # transient sbuf
            tc.tile_pool(name="ps", bufs=1, space="PSUM") as ps,
        ):
            # ---------------- persistent state ----------------
            hT = persist.tile([128, KD, TR], F32R)          # residual ^T (fp32)
            sqT = persist.tile([128, KD, TR], BF16)         # squares scratch
            hnT = persist.tile([128, KD, TC], BF16)         # LN output ^T
            oT = persist.tile([128, KD, TC], BF16)          # attn out ^T
            g1T = persist.tile([128, MD, TC], BF16)         # gelu(fc1) ^T
            KC = persist.tile([128, NQP], BF16)             # K cache ^T (x2 dup)
            VC = persist.tile([128, NKBLK, VW], BF16)       # V cache + ones col
            qpt = persist.tile([128, NQP], BF16)            # Q'^T (g,t) (x2 dup)
            otn = persist.tile([64, NQP], BF16)             # normalized O'^T
            ident = persist.tile([128, 128], F32)
            onec = persist.tile([128, 1], F32R)             # 1.0 col (LN sum lhsT)
            onecb = persist.tile([128, 1], BF16)
            oner = persist.tile([1, 128], F32R)             # 1.0 row (bcast lhsT)
            orow = persist.tile([1, NCLS], F32)

            nc.gpsimd.dma_start(
                hT, d["_zeros"].ap().rearrange("(o c) -> o c", o=1)
                .to_broadcast([128, KD * TR]))
            nc.vector.memset(oT, 0.0)
            nc.vector.memset(VC, 0.0)
            make_identity(nc, ident)
            nc.sync.dma_start(onec, d["_ones"].ap().rearrange("(p o) -> p o", o=1))
            nc.sync.dma_start(oner, d["_ones"].ap().rearrange("(o p) -> o p", o=1))
            nc.vector.memset(onecb, 1.0)
            nc.vector.memset(VC[:, :, HD:HD + 2], 1.0)

            # ---------------- weight loading helpers ----------------
            def load_qkvp(l):
                v = wq.tile([128, NV], F32, name="vecs", tag="vecs")
                nc.sync.dma_start(v, d["vecs"].ap()[l])
                qw_ = wq.tile([128, KD, DIM], BF16, name="qw", tag="qw")
                nc.sync.dma_start(qw_, d["qw"].ap()[l].rearrange("p (k c) -> p k c", k=KD))
                kvw_ = wq.tile([128, KD, 2 * HD], BF16, name="kvw", tag="kvw")
                nc.sync.dma_start(kvw_, d["kvw"].ap()[l].rearrange("p (k c) -> p k c", k=KD))
                ow_ = wq.tile([128, KD, DIM], BF16, name="ow", tag="ow")
                nc.sync.dma_start(ow_, d["projw"].ap()[l].rearrange("p (k c) -> p k c", k=KD))
                return v, qw_, kvw_, ow_

            def load_slabs(l):
                f1r = d["fc1w"].ap()[l].rearrange("p (k c) -> p k c", k=KD)
                f2r = d["fc2w"].ap()[l].rearrange("p (k c) -> p k c", k=MD)
                f1q, f2q = [], []
                for i in range(4):  # fc1 quarter: m-tiles 6i/4.. (768 cols each)
                    s = wsl.tile([128, KD, MLP // 4], BF16, name="f1q", tag="slab")
                    nc.sync.dma_start(s, f1r[:, :, i * (MLP // 4):(i + 1) * (MLP // 4)])
                    f1q.append(s)
                for i in range(4):  # fc2 quarter: k-tiles 6i..6i+5 (full 768 cols)
                    s = wsl.tile([128, KD, DIM], BF16, name="f2q", tag="slab")
                    nc.sync.dma_start(s, f2r[:, i * KD:(i + 1) * KD, :])
                    f2q.append(s)
                return f1q, f2q

            # ---------------- layernorm ----------------
            def ln(dst, wb, wo, bo):
                """LN over d of hT -> dst[128, KD, TC] (bf16).

                wb: [128, NV]-style tile; wo/bo: col offsets of gamma/beta."""
                nc.vector.tensor_tensor(sqT, hT, hT, op=OP.mult)
                ssum = ps.tile([1, 512], F32, name="ssum", tag="row", bufs=2)
                ssq = ps.tile([1, 512], F32, name="ssq", tag="row", bufs=2)
                for k in range(KD):
                    nc.tensor.matmul(ssum[:, 0:TR], onec, hT[:, k, :],
                                     start=(k == 0), stop=(k == KD - 1))
                for k in range(KD):
                    nc.tensor.matmul(ssq[:, 0:TR], onecb, sqT[:, k, :],
                                     start=(k == 0), stop=(k == KD - 1))
                mean = tmp.tile([1, TC], F32, name="mean", tag="mean", bufs=1)
                nc.vector.tensor_scalar(out=mean, in0=ssum[:, 0:TC],
                                        scalar1=1.0 / DIM, scalar2=None, op0=OP.mult)
                m2 = tmp.tile([1, TC], F32, name="m2", tag="m2", bufs=1)
                nc.vector.tensor_tensor(m2, mean, mean, op=OP.mult)
                var = tmp.tile([1, TC], F32, name="var", tag="var", bufs=1)
                nc.vector.scalar_tensor_tensor(
                    out=var, in0=ssq[:, 0:TC], scalar=1.0 / DIM,
                    in1=m2, op0=OP.mult, op1=OP.subtract)
                ve = tmp.tile([1, TC], F32, name="ve", tag="ve", bufs=1)
                nc.vector.tensor_scalar(out=ve, in0=var, scalar1=EPS,
                                        scalar2=None, op0=OP.add)
                sd = tmp.tile([1, TC], I32, name="sd", tag="sd", bufs=1)
                nc.vector.tensor_scalar(out=sd, in0=ve.bitcast(I32), scalar1=1,
                                        scalar2=None, op0=OP.logical_shift_right)
                nc.vector.tensor_scalar(out=sd, in0=sd, scalar1=-1,
                                        scalar2=0x5F3759DF, op0=OP.mult, op1=OP.add)
                y0 = sd.bitcast(F32)
                t_ = tmp.tile([1, TC], F32, name="nrT", tag="nrT", bufs=1)
                nc.vector.tensor_tensor(t_, y0, y0, op=OP.mult)
                nc.vector.tensor_tensor(t_, t_, ve, op=OP.mult)
                nc.vector.tensor_scalar(out=t_, in0=t_, scalar1=-0.5, scalar2=1.5,
                                        op0=OP.mult, op1=OP.add)
                rstd = tmp.tile([1, TC], F32R, name="rstd", tag="rstd", bufs=1)
                nc.vector.tensor_tensor(rstd, y0, t_, op=OP.mult)
                mr = tmp.tile([1, TC], F32R, name="mr", tag="mr", bufs=1)
                nc.vector.tensor_tensor(mr, mean, rstd, op=OP.mult)
                rstd_b = ps.tile([128, TC], F32, name="rstd_b", tag="row", bufs=2)
                mr_b = ps.tile([128, TC], F32, name="mr_b", tag="row", bufs=2)
                nc.tensor.matmul(rstd_b, oner, rstd, start=True, stop=True)
                nc.tensor.matmul(mr_b, oner, mr, start=True, stop=True)
                for k in range(KD):
                    t1 = tmp.tile([128, TC], F32, name="lnt", tag="lnt", bufs=2)
                    nc.vector.tensor_tensor(t1, hT[:, k, 0:TC], rstd_b, op=OP.mult)
                    nc.vector.tensor_tensor(t1, t1, mr_b, op=OP.subtract)
                    nc.vector.tensor_scalar(
                        out=dst[:, k, :], in0=t1,
                        scalar1=wb[:, wo + k:wo + k + 1],
                        scalar2=wb[:, bo + k:bo + k + 1],
                        op0=OP.mult, op1=OP.add)

            # ---------------- prologue: patch embed ----------------
            xpt = tmp.tile([128, KD, TC], BF16, name="xpt", tag="pt", bufs=2)
            nc.sync.dma_start(xpt, d["xpt"].ap().rearrange("p (k c) -> p k c", k=KD))
            posbt = tmp.tile([128, KD, TC], F32, name="posbt", tag="posbt", bufs=1)
            nc.sync.dma_start(posbt, d["posbt"].ap().rearrange("p (k c) -> p k c", k=KD))
            pw_sb = wq.tile([128, KD, DIM], BF16, name="pw_sb", tag="qw")
            nc.sync.dma_start(pw_sb, d["patchw"].ap().rearrange("p (k c) -> p k c", k=KD))
            vecs0 = load_qkvp(0)
            slabs0 = load_slabs(0)

            for m in range(KD):
                pp = ps.tile([128, 1024], F32, name="pp", tag="big", bufs=3)
                for k in range(KD):
                    nc.tensor.matmul(pp[:, 0:TC], pw_sb[:, k, m * 128:(m + 1) * 128],
                                     xpt[:, k, :], start=(k == 0), stop=(k == KD - 1))
                nc.vector.tensor_tensor(hT[:, m, 0:TC], pp[:, 0:TC],
                                        posbt[:, m, :], op=OP.add)

            # ---------------- transformer layers ----------------
            lw = (vecs0, slabs0)
            for l in range(DEPTH):
                (vv, qw_sb, kvw_sb, ow_sb), (f1q, f2q) = lw
                if l + 1 < DEPTH:
                    nxt = (load_qkvp(l + 1), load_slabs(l + 1))

                ln(hnT, vv, VO_L1W, VO_L1B)

                # ---- Q projection -> qpt strips ----
                for m in range(KD):
                    qp = ps.tile([128, 1024], F32, name="qp", tag="big", bufs=3)
                    for k in range(KD):
                        nc.tensor.matmul(qp[:, 0:TC], qw_sb[:, k, m * 128:(m + 1) * 128],
                                         hnT[:, k, :], start=(k == 0), stop=(k == KD - 1))
                    for dst0 in (0, 64):
                        nc.vector.tensor_scalar(
                            out=qpt[dst0:dst0 + 64, (2 * m) * T:(2 * m) * T + T],
                            in0=qp[0:64, 0:T],
                            scalar1=vv[0:64, VO_QB + m:VO_QB + m + 1], scalar2=None,
                            op0=OP.add)
                        nc.vector.tensor_scalar(
                            out=qpt[dst0:dst0 + 64, (2 * m + 1) * T:(2 * m + 1) * T + T],
                            in0=qp[64:128, 0:T],
                            scalar1=vv[64:128, VO_QB + m:VO_QB + m + 1], scalar2=None,
                            op0=OP.add)

                # ---- KV projection; append K^T and V to caches ----
                kvp = ps.tile([128, 1024], F32, name="kvp", tag="big", bufs=3)
                for k in range(KD):
                    nc.tensor.matmul(kvp[:, 0:TC], kvw_sb[:, k, :], hnT[:, k, :],
                                     start=(k == 0), stop=(k == KD - 1))
                for dst0 in (0, 64):
                    nc.vector.tensor_scalar(
                        out=KC[dst0:dst0 + 64, l * T:l * T + T], in0=kvp[0:64, 0:T],
                        scalar1=vv[0:64, VO_KVB:VO_KVB + 1], scalar2=None, op0=OP.add)
                vsb = tmp.tile([128, TC], F32, name="vsb", tag="vsb", bufs=1)
                nc.vector.tensor_scalar(
                    out=vsb[64:128, :], in0=kvp[64:128, 0:TC],
                    scalar1=vv[64:128, VO_KVB:VO_KVB + 1], scalar2=None, op0=OP.add)
                for tc_i, tsz in ((0, 128), (1, 69)):
                    vtp = ps.tile([128, 512], F32, name="vtp", tag="row", bufs=2)
                    nc.tensor.matmul(vtp[0:tsz, 0:HD],
                                     vsb[64:128, tc_i * 128:tc_i * 128 + tsz],
                                     ident[64:128, 64:64 + HD], is_transpose=True,
                                     start=True, stop=True)
                    vts = tmp.tile([128, HD], BF16, name="vts", tag="vts", bufs=2)
                    nc.vector.tensor_copy(vts[0:tsz], vtp[0:tsz, 0:HD])
                    t0 = 0
                    while t0 < tsz:
                        kpos = l * T + tc_i * 128 + t0
                        blk, off = kpos // 128, kpos % 128
                        cnt = min(tsz - t0, 128 - off)
                        nc.scalar.dma_start(
                            VC[off:off + cnt, blk, 0:HD],
                            vts[t0:t0 + cnt, :])
                        t0 += cnt

                # ---- attention ----
                Lk = (l + 1) * T
                nkt = (Lk + 127) // 128
                for qoff, qsz in QP_:
                    nh = (qsz + 511) // 512
                    ot = ps.tile([128, 1024], F32, name="ot", tag="big", bufs=3)
                    for c in range(nkt):
                        ksz = min(128, Lk - c * 128)
                        st = ps.tile([128, 1024], F32, name="st", tag="big", bufs=3)
                        for h in range(nh):
                            cw = min(512, qsz - h * 512)
                            nc.tensor.matmul(
                                st[0:ksz, h * 512:h * 512 + cw],
                                KC[:, c * 128:c * 128 + ksz],
                                qpt[:, qoff + h * 512:qoff + h * 512 + cw],
                                start=True, stop=True)
                        pt = tmp.tile([128, 1024], BF16, name="pt", tag="pt", bufs=2)
                        nc.scalar.activation(pt[0:ksz, 0:qsz], st[0:ksz, 0:qsz],
                                             AF.Exp, scale=SCALE / 2)
                        for h in range(nh):
                            cw = min(512, qsz - h * 512)
                            nc.tensor.matmul(
                                ot[:, h * 512:h * 512 + cw],
                                VC[0:ksz, c, 0:128],
                                pt[0:ksz, h * 512:h * 512 + cw],
                                start=(c == 0), stop=(c == nkt - 1))
                    # normalize: otn = ot[0:64] * (1/den) broadcast
                    for h in range(nh):
                        cw = min(512, qsz - h * 512)
                        denr = tmp.tile([1, 512], F32R, name="denr", tag="rec", bufs=2)
                        nc.vector.tensor_copy(denr[:, 0:cw],
                                              ot[64:65, h * 512:h * 512 + cw])
                        rbp = ps.tile([64, 512], F32, name="rbp", tag="row", bufs=2)
                        nc.tensor.matmul(rbp[:, 0:cw], oner[:, 0:64], denr[:, 0:cw],
                                         start=True, stop=True)
                        rb = tmp.tile([64, 512], F32, name="rb", tag="rb", bufs=2)
                        nc.vector.reciprocal_approx_fast(rb[:, 0:cw], rbp[:, 0:cw])
                        nc.vector.tensor_tensor(
                            otn[:, qoff + h * 512:qoff + h * 512 + cw],
                            ot[0:64, h * 512:h * 512 + cw], rb[:, 0:cw],
                            op=OP.mult)

                # ---- reshape O'T (g,t) -> oT [d, t] ----
                for g in range(G):
                    j, half = g // 2, g % 2
                    nc.vector.tensor_copy(oT[64 * half:64 * half + 64, j, 0:T],
                                          otn[:, g * T:g * T + T])

                # ---- output projection + residual ----
                for m in range(KD):
                    op_ = ps.tile([128, 1024], F32, name="prp", tag="big", bufs=3)
                    for k in range(KD):
                        nc.tensor.matmul(op_[:, 0:TC], ow_sb[:, k, m * 128:(m + 1) * 128],
                                         oT[:, k, :], start=(k == 0), stop=(k == KD - 1))
                    nc.vector.scalar_tensor_tensor(
                        out=hT[:, m, 0:T], in0=op_[:, 0:T],
                        scalar=vv[:, VO_PB + m:VO_PB + m + 1],
                        in1=hT[:, m, 0:T], op0=OP.add, op1=OP.add)

                # ---- MLP ----
                ln(hnT, vv, VO_L2W, VO_L2B)
                for m in range(MD):
                    f1s = f1q[m // 6]
                    mi = m % 6
                    fp = ps.tile([128, 1024], F32, name="fp", tag="big", bufs=3)
                    for k in range(KD):
                        nc.tensor.matmul(fp[:, 0:TC], f1s[:, k, mi * 128:(mi + 1) * 128],
                                         hnT[:, k, :], start=(k == 0), stop=(k == KD - 1))
                    nc.scalar.activation(g1T[:, m, :], fp[:, 0:TC], AF.Gelu,
                                         bias=vv[:, VO_F1B + m:VO_F1B + m + 1])
                for m in range(KD):
                    f2p = ps.tile([128, 1024], F32, name="f2p", tag="big", bufs=3)
                    for k in range(MD):
                        f2s = f2q[k // 6]
                        nc.tensor.matmul(f2p[:, 0:TC],
                                         f2s[:, k % 6, m * 128:(m + 1) * 128],
                                         g1T[:, k, :], start=(k == 0), stop=(k == MD - 1))
                    nc.vector.scalar_tensor_tensor(
                        out=hT[:, m, 0:T], in0=f2p[:, 0:T],
                        scalar=vv[:, VO_F2B + m:VO_F2B + m + 1],
                        in1=hT[:, m, 0:T], op0=OP.add, op1=OP.add)

                if l + 1 < DEPTH:
                    lw = nxt

            # ---------------- final LN + head ----------------
            nv = persist.tile([128, 12], F32)
            nc.sync.dma_start(nv, d["normv"].ap())
            nc.sync.dma_start(orow, d["headb"].ap().rearrange("(o c) -> o c", o=1))
            ln(hnT, nv, 0, 6)
            hwr = d["headw"].ap().rearrange("p (k c) -> p k c", k=KD)
            for n in range(2):
                hw_c = wsl.tile([128, KD, 500], BF16, name="hw_c", tag="slab")
                nc.sync.dma_start(hw_c, hwr[:, :, n * 500:(n + 1) * 500])
                hp = ps.tile([1, 512], F32, name="hp", tag="row", bufs=2)
                for k in range(KD):
                    nc.tensor.matmul(hp[:, 0:500], hnT[:, k, 0:1], hw_c[:, k, :],
                                     start=(k == 0), stop=(k == KD - 1))
                nc.vector.tensor_tensor(orow[:, n * 500:(n + 1) * 500], hp[:, 0:500],
                                        orow[:, n * 500:(n + 1) * 500], op=OP.add)
            nc.sync.dma_start(out_d.ap(), orow)

    nc.compile()
    return nc


def _tile_w(w):
    """(K*128, C) fp32 -> (128, K*C) bf16 tiled: out[p, k*C+c] = w[k*128+p, c]."""
    k = w.shape[0] // 128
    c = w.shape[1]
    return np.ascontiguousarray(
        w.reshape(k, 128, c).transpose(1, 0, 2).reshape(128, k * c).astype(BFNP))


def _vcol(v):
    """(K*128,) -> (128, K): out[p, k] = v[k*128+p]."""
    k = v.shape[0] // 128
    return v.reshape(k, 128).T


def make_in_maps(inputs):
    f = {n: np.asarray(inputs[n], dtype=np.float32) for n in inputs}

    shared = {}
    shared["patchw"] = _tile_w(f["patch_w"])
    shared["qw"] = np.stack([_tile_w(f["q_w"][l]) for l in range(DEPTH)])
    shared["kvw"] = np.stack([_tile_w(f["kv_w"][l]) for l in range(DEPTH)])
    shared["projw"] = np.stack([_tile_w(f["proj_w"][l]) for l in range(DEPTH)])
    shared["fc1w"] = np.stack([_tile_w(f["fc1_w"][l]) for l in range(DEPTH)])
    shared["fc2w"] = np.stack([_tile_w(f["fc2_w"][l]) for l in range(DEPTH)])
    shared["headw"] = _tile_w(f["head_w"])
    shared["headb"] = f["head_b"]

    vecs = np.zeros((DEPTH, 128, NV), np.float32)
    for l in range(DEPTH):
        vecs[l, :, VO_L1W:VO_L1W + 6] = _vcol(f["ln1_w"][l])
        vecs[l, :, VO_L1B:VO_L1B + 6] = _vcol(f["ln1_b"][l])
        vecs[l, :, VO_QB:VO_QB + 6] = _vcol(f["q_b"][l])
        vecs[l, :, VO_KVB] = f["kv_b"][l]
        vecs[l, :, VO_PB:VO_PB + 6] = _vcol(f["proj_b"][l])
        vecs[l, :, VO_L2W:VO_L2W + 6] = _vcol(f["ln2_w"][l])
        vecs[l, :, VO_L2B:VO_L2B + 6] = _vcol(f["ln2_b"][l])
        vecs[l, :, VO_F1B:VO_F1B + 24] = _vcol(f["fc1_b"][l])
        vecs[l, :, VO_F2B:VO_F2B + 6] = _vcol(f["fc2_b"][l])
    shared["vecs"] = np.ascontiguousarray(vecs)

    normv = np.zeros((128, 12), np.float32)
    normv[:, 0:6] = _vcol(f["norm_w"])
    normv[:, 6:12] = _vcol(f["norm_b"])
    shared["normv"] = normv

    # pos_embed + patch_b / cls folding, transposed token layout
    posb = np.zeros((DIM, TC), np.float32)
    posb[:, 0] = f["cls_token"][0, 0] + f["pos_embed"][0, 0]
    posb[:, 1:T] = (f["pos_embed"][0, 1:T] + f["patch_b"][None, :]).T
    shared["posbt"] = np.ascontiguousarray(
        posb.reshape(KD, 128, TC).transpose(1, 0, 2).reshape(128, KD * TC))

    shared["_ones"] = np.ones((128,), np.float32)
    shared["_zeros"] = np.zeros((KD * TR,), np.float32)

    # per-core im2col (transposed): xpt[(c,a,b), 1 + i*14 + j]
    HG = IMG // P
    x = np.asarray(inputs["x"], dtype=np.float32)
    maps = []
    for b in range(B):
        xp = x[b].reshape(CIN, HG, P, HG, P).transpose(0, 2, 4, 1, 3)
        xp = xp.reshape(DIM, NPATCH)
        xt = np.zeros((DIM, TC), np.float32)
        xt[:, 1:T] = xp
        xt = xt.reshape(KD, 128, TC).transpose(1, 0, 2).reshape(128, KD * TC)
        maps.append(dict(shared, xpt=np.ascontiguousarray(xt.astype(BFNP))))
    return maps


def kernel(**inputs):
    if "nc" not in _CACHED:
        _CACHED["nc"] = build_module()
    nc = _CACHED["nc"]
    res = run_bass_kernel_spmd(nc, make_in_maps(inputs), core_ids=list(range(B)))
    return np.concatenate([res.results[b]["out"] for b in range(B)], axis=0)


# revision 25
# speedup vs baseline: 1.9138x; 1.0201x over previous
"""MoDA Vision Transformer forward pass on 8 Trainium2 NeuronCores.

Sharding: pure data-parallel over batch (B=8 -> 1 image per core, weights
replicated, no collectives).

v2 design (bf16 compute, fp32 residual):
- All weights pre-tiled + cast to bf16 on HOST into exactly the SBUF layout
  [p, k, c], so every weight DMA is one contiguous chunk per partition
  (128 descriptors per load instead of ~10k strided ones).
- Residual stream hT stays fp32 (f32r) padded to 256 token cols so the
  LayerNorm sum matmuls run at full f32r rate; all other activations are
  bf16 at 208 token cols (bf16 matmuls are full rate at any width).
- im2col + pos_embed/cls/bias folding done on host; patch embed is a
  plain linear directly into the transposed residual layout.
- Attention (GQA kv_heads=1, depth-KV cache): q pieces of (1024,1024,316)
  flattened (g,t) queries; per kpos-block one 1024-wide exp on ACT
  (amortizes the 352-cycle ACTIVATE overhead); denominators via a ones
  column appended to V; softmax normalize with DVE reciprocal.
- PSUM: 3x [128,1024] "big" slots (6 banks) + 2x [1-128,512] "row" slots
  (2 banks) = exactly 8 banks.
- Next layer's weights are prefetched at the top of each layer body on the
  sync HWDGE ring; small strip DMAs ride the scalar HWDGE ring.
"""

import numpy as np
import ml_dtypes

import concourse.bass as bass
import concourse.mybir as mybir
import concourse.tile as tile
from concourse import bacc
from concourse.bass_utils import run_bass_kernel_spmd
from concourse.masks import make_identity

F32 = mybir.dt.float32
F32R = mybir.dt.float32r
BF16 = mybir.dt.bfloat16
I32 = mybir.dt.int32
AF = mybir.ActivationFunctionType
OP = mybir.AluOpType
BFNP = ml_dtypes.bfloat16

B, CIN, IMG, P = 8, 3, 224, 16
DIM, DEPTH, NH, NKV = 768, 12, 12, 1
HD = DIM // NH              # 64
G = NH // NKV               # 12
NPATCH = (IMG // P) ** 2    # 196
T = NPATCH + 1              # 197
TC = 208                    # bf16 activation token cols (197 padded)
TR = 256                    # fp32 residual token cols (f32r full-rate >=256)
KD = DIM // 128             # 6
MLP = 4 * DIM               # 3072
MD = MLP // 128             # 24
NQ = G * T                  # 2364
NQP = 2368                  # padded
NKBLK = (DEPTH * T + 127) // 128  # 19
VW = 130                    # V block width (64 V + 1 ones + 63 zero + pad)
SCALE = HD ** -0.5
EPS = 1e-6
NCLS = 1000
QP_ = [(0, 1024), (1024, 1024), (2048, NQ - 2048)]  # q pieces

# packed per-layer vector columns (fp32): [128, NV]
VO_L1W, VO_L1B, VO_QB, VO_KVB, VO_PB = 0, 6, 12, 18, 19
VO_L2W, VO_L2B, VO_F1B, VO_F2B, NV = 25, 31, 37, 61, 67

_CACHED = {}


def build_module():
    nc = bacc.Bacc("TRN2", target_bir_lowering=False, debug=False, num_devices=8)

    d = {}
    def din(name, shape, dt_):
        d[name] = nc.dram_tensor(name, shape, dt_, kind="ExternalInput")

    din("xpt", (128, KD * TC), BF16)
    din("posbt", (128, KD * TC), F32)
    din("patchw", (128, KD * DIM), BF16)
    din("qw", (DEPTH, 128, KD * DIM), BF16)
    din("kvw", (DEPTH, 128, KD * 2 * HD), BF16)
    din("projw", (DEPTH, 128, KD * DIM), BF16)
    din("fc1w", (DEPTH, 128, KD * MLP), BF16)
    din("fc2w", (DEPTH, 128, MD * DIM), BF16)
    din("vecs", (DEPTH, 128, NV), F32)
    din("normv", (128, 12), F32)
    din("headw", (128, KD * NCLS), BF16)
    din("headb", (NCLS,), F32)
    din("_ones", (128,), F32R)
    din("_zeros", (KD * TR,), F32R)
    out_d = nc.dram_tensor("out", (1, NCLS), F32, kind="ExternalOutput")

    with tile.TileContext(nc) as tc:
        with (
            tc.tile_pool(name="persist", bufs=1) as persist,
            tc.tile_pool(name="wq", bufs=2) as wq,        # q/kv/proj + vecs
            tc.tile_pool(name="wsl", bufs=10) as wsl,     # fc1/fc2 quarter slabs
            tc.tile_pool(name="tmp", bufs=2) as tmp,      # transient sbuf
            tc.tile_pool(name="ps", bufs=1, space="PSUM") as ps,
        ):
            # ---------------- persistent state ----------------
            hT = persist.tile([128, KD, TR], F32R)          # residual ^T (fp32)
            sqT = persist.tile([128, KD, TR], BF16)         # squares scratch
            hnT = persist.tile([128, KD, TC], BF16)         # LN output ^T
            oT = persist.tile([128, KD, TC], BF16)          # attn out ^T
            g1T = persist.tile([128, MD, TC], BF16)         # gelu(fc1) ^T
            KC = persist.tile([128, NQP], BF16)             # K cache ^T (x2 dup)
            VC = persist.tile([128, NKBLK, VW], BF16)       # V cache + ones col
            qpt = persist.tile([128, NQP], BF16)            # Q'^T (g,t) (x2 dup)
            otn = persist.tile([64, NQP], BF16)             # normalized O'^T
            ident = persist.tile([128, 128], F32)
            onec = persist.tile([128, 1], F32R)             # 1.0 col (LN sum lhsT)
            onecb = persist.tile([128, 1], BF16)
            oner = persist.tile([1, 128], F32R)             # 1.0 row (bcast lhsT)
            orow = persist.tile([1, NCLS], F32)

            nc.gpsimd.dma_start(
                hT, d["_zeros"].ap().rearrange("(o c) -> o c", o=1)
                .to_broadcast([128, KD * TR]))
            nc.vector.memset(oT, 0.0)
            nc.vector.memset(VC, 0.0)
            make_identity(nc, ident)
            nc.sync.dma_start(onec, d["_ones"].ap().rearrange("(p o) -> p o", o=1))
            nc.sync.dma_start(oner, d["_ones"].ap().rearrange("(o p) -> o p", o=1))
            nc.vector.memset(onecb, 1.0)
            nc.vector.memset(VC[:, :, HD:HD + 2], 1.0)

            # ---------------- weight loading helpers ----------------
            def load_qkvp(l):
                v = wq.tile([128, NV], F32, name="vecs", tag="vecs")
                nc.sync.dma_start(v, d["vecs"].ap()[l])
                qw_ = wq.tile([128, KD, DIM], BF16, name="qw", tag="qw")
                nc.sync.dma_start(qw_, d["qw"].ap()[l].rearrange("p (k c) -> p k c", k=KD))
                kvw_ = wq.tile([128, KD, 2 * HD], BF16, name="kvw", tag="kvw")
                nc.sync.dma_start(kvw_, d["kvw"].ap()[l].rearrange("p (k c) -> p k c", k=KD))
                ow_ = wq.tile([128, KD, DIM], BF16, name="ow", tag="ow")
                nc.sync.dma_start(ow_, d["projw"].ap()[l].rearrange("p (k c) -> p k c", k=KD))
                return v, qw_, kvw_, ow_

            def load_slabs(l):
                f1r = d["fc1w"].ap()[l].rearrange("p (k c) -> p k c", k=KD)
                f2r = d["fc2w"].ap()[l].rearrange("p (k c) -> p k c", k=MD)
                f1q, f2q = [], []
                for i in range(4):  # fc1 quarter: m-tiles 6i/4.. (768 cols each)
                    s = wsl.tile([128, KD, MLP // 4], BF16, name="f1q", tag="slab")
                    nc.sync.dma_start(s, f1r[:, :, i * (MLP // 4):(i + 1) * (MLP // 4)])
                    f1q.append(s)
                for i in range(4):  # fc2 quarter: k-tiles 6i..6i+5 (full 768 cols)
                    s = wsl.tile([128, KD, DIM], BF16, name="f2q", tag="slab")
                    nc.sync.dma_start(s, f2r[:, i * KD:(i + 1) * KD, :])
                    f2q.append(s)
                return f1q, f2q

            # ---------------- layernorm ----------------
            def ln(dst, wb, wo, bo):
                """LN over d of hT -> dst[128, KD, TC] (bf16).

                wb: [128, NV]-style tile; wo/bo: col offsets of gamma/beta."""
                nc.vector.tensor_tensor(sqT, hT, hT, op=OP.mult)
                ssum = ps.tile([1, 512], F32, name="ssum", tag="row", bufs=2)
                ssq = ps.tile([1, 512], F32, name="ssq", tag="row", bufs=2)
                for k in range(KD):
                    nc.tensor.matmul(ssum[:, 0:TR], onec, hT[:, k, :],
                                     start=(k == 0), stop=(k == KD - 1))
                for k in range(KD):
                    nc.tensor.matmul(ssq[:, 0:TR], onecb, sqT[:, k, :],
                                     start=(k == 0), stop=(k == KD - 1))
                mean = tmp.tile([1, TC], F32, name="mean", tag="mean", bufs=1)
                nc.vector.tensor_scalar(out=mean, in0=ssum[:, 0:TC],
                                        scalar1=1.0 / DIM, scalar2=None, op0=OP.mult)
                m2 = tmp.tile([1, TC], F32, name="m2", tag="m2", bufs=1)
                nc.vector.tensor_tensor(m2, mean, mean, op=OP.mult)
                var = tmp.tile([1, TC], F32, name="var", tag="var", bufs=1)
                nc.vector.scalar_tensor_tensor(
                    out=var, in0=ssq[:, 0:TC], scalar=1.0 / DIM,
                    in1=m2, op0=OP.mult, op1=OP.subtract)
                ve = tmp.tile([1, TC], F32, name="ve", tag="ve", bufs=1)
                nc.vector.tensor_scalar(out=ve, in0=var, scalar1=EPS,
                                        scalar2=None, op0=OP.add)
                sd = tmp.tile([1, TC], I32, name="sd", tag="sd", bufs=1)
                nc.vector.tensor_scalar(out=sd, in0=ve.bitcast(I32), scalar1=1,
                                        scalar2=None, op0=OP.logical_shift_right)
                nc.vector.tensor_scalar(out=sd, in0=sd, scalar1=-1,
                                        scalar2=0x5F3759DF, op0=OP.mult, op1=OP.add)
                y0 = sd.bitcast(F32)
                t_ = tmp.tile([1, TC], F32, name="nrT", tag="nrT", bufs=1)
                nc.vector.tensor_tensor(t_, y0, y0, op=OP.mult)
                nc.vector.tensor_tensor(t_, t_, ve, op=OP.mult)
                nc.vector.tensor_scalar(out=t_, in0=t_, scalar1=-0.5, scalar2=1.5,
                                        op0=OP.mult, op1=OP.add)
                rstd = tmp.tile([1, TC], F32R, name="rstd", tag="rstd", bufs=1)
                nc.vector.tensor_tensor(rstd, y0, t_, op=OP.mult)
                mr = tmp.tile([1, TC], F32R, name="mr", tag="mr", bufs=1)
                nc.vector.tensor_tensor(mr, mean, rstd, op=OP.mult)
                for _ in range(8):
                    dmy = ps.tile([128, 512], F32, name="dmyl", tag="row", bufs=2)
                    nc.tensor.matmul(dmy, KC[:, 0:128], qpt[:, 0:512],
                                     start=True, stop=True)
                rstd_b = ps.tile([128, TC], F32, name="rstd_b", tag="row", bufs=2)
                mr_b = ps.tile([128, TC], F32, name="mr_b", tag="row", bufs=2)
                nc.tensor.matmul(rstd_b, oner, rstd, start=True, stop=True)
                nc.tensor.matmul(mr_b, oner, mr, start=True, stop=True)
                for k in range(KD):
                    t1 = tmp.tile([128, TC], F32, name="lnt", tag="lnt", bufs=2)
                    nc.vector.tensor_tensor(t1, hT[:, k, 0:TC], rstd_b, op=OP.mult)
                    nc.vector.tensor_tensor(t1, t1, mr_b, op=OP.subtract)
                    nc.vector.tensor_scalar(
                        out=dst[:, k, :], in0=t1,
                        scalar1=wb[:, wo + k:wo + k + 1],
                        scalar2=wb[:, bo + k:bo + k + 1],
                        op0=OP.mult, op1=OP.add)

            # ---------------- prologue: patch embed ----------------
            xpt = tmp.tile([128, KD, TC], BF16, name="xpt", tag="pt", bufs=2)
            nc.sync.dma_start(xpt, d["xpt"].ap().rearrange("p (k c) -> p k c", k=KD))
            posbt = wsl.tile([128, KD, TC], F32, name="posbt", tag="slab")
            nc.sync.dma_start(posbt, d["posbt"].ap().rearrange("p (k c) -> p k c", k=KD))
            pw_sb = wq.tile([128, KD, DIM], BF16, name="pw_sb", tag="qw")
            nc.sync.dma_start(pw_sb, d["patchw"].ap().rearrange("p (k c) -> p k c", k=KD))
            vecs0 = load_qkvp(0)
            slabs0 = load_slabs(0)

            for m in range(KD):
                pp = ps.tile([128, 1024], F32, name="pp", tag="big", bufs=3)
                for k in range(KD):
                    nc.tensor.matmul(pp[:, 0:TC], pw_sb[:, k, m * 128:(m + 1) * 128],
                                     xpt[:, k, :], start=(k == 0), stop=(k == KD - 1))
                nc.vector.tensor_tensor(hT[:, m, 0:TC], pp[:, 0:TC],
                                        posbt[:, m, :], op=OP.add)

            # ---------------- transformer layers ----------------
            lw = (vecs0, slabs0)
            for l in range(DEPTH):
                (vv, qw_sb, kvw_sb, ow_sb), (f1q, f2q) = lw
                if l + 1 < DEPTH:
                    nxt = (load_qkvp(l + 1), load_slabs(l + 1))

                ln(hnT, vv, VO_L1W, VO_L1B)

                # ---- Q projection -> qpt strips ----
                for m in range(KD):
                    qp = ps.tile([128, 1024], F32, name="qp", tag="big", bufs=3)
                    for k in range(KD):
                        nc.tensor.matmul(qp[:, 0:TC], qw_sb[:, k, m * 128:(m + 1) * 128],
                                         hnT[:, k, :], start=(k == 0), stop=(k == KD - 1))
                    for dst0 in (0, 64):
                        nc.vector.tensor_scalar(
                            out=qpt[dst0:dst0 + 64, (2 * m) * T:(2 * m) * T + T],
                            in0=qp[0:64, 0:T],
                            scalar1=vv[0:64, VO_QB + m:VO_QB + m + 1], scalar2=None,
                            op0=OP.add)
                        nc.vector.tensor_scalar(
                            out=qpt[dst0:dst0 + 64, (2 * m + 1) * T:(2 * m + 1) * T + T],
                            in0=qp[64:128, 0:T],
                            scalar1=vv[64:128, VO_QB + m:VO_QB + m + 1], scalar2=None,
                            op0=OP.add)

                # ---- KV projection; append K^T and V to caches ----
                kvp = ps.tile([128, 1024], F32, name="kvp", tag="big", bufs=3)
                for k in range(KD):
                    nc.tensor.matmul(kvp[:, 0:TC], kvw_sb[:, k, :], hnT[:, k, :],
                                     start=(k == 0), stop=(k == KD - 1))
                for dst0 in (0, 64):
                    nc.vector.tensor_scalar(
                        out=KC[dst0:dst0 + 64, l * T:l * T + T], in0=kvp[0:64, 0:T],
                        scalar1=vv[0:64, VO_KVB:VO_KVB + 1], scalar2=None, op0=OP.add)
                vsb = tmp.tile([128, TC], F32, name="vsb", tag="vsb", bufs=1)
                nc.vector.tensor_scalar(
                    out=vsb[64:128, :], in0=kvp[64:128, 0:TC],
                    scalar1=vv[64:128, VO_KVB:VO_KVB + 1], scalar2=None, op0=OP.add)
                for tc_i, tsz in ((0, 128), (1, 69)):
                    vtp = ps.tile([128, 512], F32, name="vtp", tag="row", bufs=2)
                    nc.tensor.matmul(vtp[0:tsz, 0:HD],
                                     vsb[64:128, tc_i * 128:tc_i * 128 + tsz],
                                     ident[64:128, 64:64 + HD], is_transpose=True,
                                     start=True, stop=True)
                    vts = tmp.tile([128, HD], BF16, name="vts", tag="vts", bufs=2)
                    nc.vector.tensor_copy(vts[0:tsz], vtp[0:tsz, 0:HD])
                    t0 = 0
                    while t0 < tsz:
                        kpos = l * T + tc_i * 128 + t0
                        blk, off = kpos // 128, kpos % 128
                        cnt = min(tsz - t0, 128 - off)
                        nc.scalar.dma_start(
                            VC[off:off + cnt, blk, 0:HD],
                            vts[t0:t0 + cnt, :])
                        t0 += cnt

                # ---- attention ----
                Lk = (l + 1) * T
                nkt = (Lk + 127) // 128
                for qoff, qsz in QP_:
                    nh = (qsz + 511) // 512
                    ot = ps.tile([128, 1024], F32, name="ot", tag="big", bufs=3)
                    for c in range(nkt):
                        ksz = min(128, Lk - c * 128)
                        st = ps.tile([128, 1024], F32, name="st", tag="big", bufs=3)
                        for h in range(nh):
                            cw = min(512, qsz - h * 512)
                            nc.tensor.matmul(
                                st[0:ksz, h * 512:h * 512 + cw],
                                KC[:, c * 128:c * 128 + ksz],
                                qpt[:, qoff + h * 512:qoff + h * 512 + cw],
                                start=True, stop=True)
                        dmy = ps.tile([128, 512], F32, name="dmy", tag="row",
                                      bufs=2)
                        nc.tensor.matmul(dmy, KC[:, 0:128], qpt[:, 0:512],
                                         start=True, stop=True)
                        pt = tmp.tile([128, 1024], BF16, name="pt", tag="pt", bufs=2)
                        nc.scalar.activation(pt[0:ksz, 0:qsz], st[0:ksz, 0:qsz],
                                             AF.Exp, scale=SCALE / 2)
                        for h in range(nh):
                            cw = min(512, qsz - h * 512)
                            nc.tensor.matmul(
                                ot[:, h * 512:h * 512 + cw],
                                VC[0:ksz, c, 0:128],
                                pt[0:ksz, h * 512:h * 512 + cw],
                                start=(c == 0), stop=(c == nkt - 1))
                    # evacuate ot fast (frees the PSUM slot for the next
                    # piece), then normalize off the critical path
                    ots = tmp.tile([65, 1024], F32, name="ots", tag="ots", bufs=2)
                    nc.vector.tensor_copy(ots[:, 0:qsz], ot[0:65, 0:qsz])
                    for h in range(nh):
                        cw = min(512, qsz - h * 512)
                        denr = tmp.tile([1, 512], F32R, name="denr", tag="rec", bufs=2)
                        nc.vector.tensor_copy(denr[:, 0:cw],
                                              ots[64:65, h * 512:h * 512 + cw])
                        rbp = ps.tile([64, 512], F32, name="rbp", tag="row", bufs=2)
                        nc.tensor.matmul(rbp[:, 0:cw], oner[:, 0:64], denr[:, 0:cw],
                                         start=True, stop=True)
                        rb = tmp.tile([64, 512], F32, name="rb", tag="rb", bufs=2)
                        nc.vector.reciprocal_approx_fast(rb[:, 0:cw], rbp[:, 0:cw])
                        nc.vector.tensor_tensor(
                            otn[:, qoff + h * 512:qoff + h * 512 + cw],
                            ots[0:64, h * 512:h * 512 + cw], rb[:, 0:cw],
                            op=OP.mult)

                # ---- reshape O'T (g,t) -> oT [d, t] ----
                for g in range(G):
                    j, half = g // 2, g % 2
                    nc.vector.tensor_copy(oT[64 * half:64 * half + 64, j, 0:T],
                                          otn[:, g * T:g * T + T])

                # ---- output projection + residual ----
                for m in range(KD):
                    op_ = ps.tile([128, 1024], F32, name="prp", tag="big", bufs=3)
                    for k in range(KD):
                        nc.tensor.matmul(op_[:, 0:TC], ow_sb[:, k, m * 128:(m + 1) * 128],
                                         oT[:, k, :], start=(k == 0), stop=(k == KD - 1))
                    nc.vector.scalar_tensor_tensor(
                        out=hT[:, m, 0:T], in0=op_[:, 0:T],
                        scalar=vv[:, VO_PB + m:VO_PB + m + 1],
                        in1=hT[:, m, 0:T], op0=OP.add, op1=OP.add)

                # ---- MLP ----
                ln(hnT, vv, VO_L2W, VO_L2B)
                for m in range(MD):
                    f1s = f1q[m // 6]
                    mi = m % 6
                    fp = ps.tile([128, 1024], F32, name="fp", tag="big", bufs=3)
                    for k in range(KD):
                        nc.tensor.matmul(fp[:, 0:TC], f1s[:, k, mi * 128:(mi + 1) * 128],
                                         hnT[:, k, :], start=(k == 0), stop=(k == KD - 1))
                    nc.scalar.activation(g1T[:, m, :], fp[:, 0:TC], AF.Gelu,
                                         bias=vv[:, VO_F1B + m:VO_F1B + m + 1])
                for m in range(KD):
                    f2p = ps.tile([128, 1024], F32, name="f2p", tag="big", bufs=3)
                    for k in range(MD):
                        f2s = f2q[k // 6]
                        nc.tensor.matmul(f2p[:, 0:TC],
                                         f2s[:, k % 6, m * 128:(m + 1) * 128],
                                         g1T[:, k, :], start=(k == 0), stop=(k == MD - 1))
                    nc.vector.scalar_tensor_tensor(
                        out=hT[:, m, 0:T], in0=f2p[:, 0:T],
                        scalar=vv[:, VO_F2B + m:VO_F2B + m + 1],
                        in1=hT[:, m, 0:T], op0=OP.add, op1=OP.add)

                if l + 1 < DEPTH:
                    lw = nxt

            # ---------------- final LN + head ----------------
            nv = persist.tile([128, 12], F32)
            nc.sync.dma_start(nv, d["normv"].ap())
            nc.sync.dma_start(orow, d["headb"].ap().rearrange("(o c) -> o c", o=1))
            ln(hnT, nv, 0, 6)
            hwr = d["headw"].ap().rearrange("p (k c) -> p k c", k=KD)
            for n in range(2):
                hw_c = wsl.tile([128, KD, 500], BF16, name="hw_c", tag="slab")
                nc.sync.dma_start(hw_c, hwr[:, :, n * 500:(n + 1) * 500])
                hp = ps.tile([1, 512], F32, name="hp", tag="row", bufs=2)
                for k in range(KD):
                    nc.tensor.matmul(hp[:, 0:500], hnT[:, k, 0:1], hw_c[:, k, :],
                                     start=(k == 0), stop=(k == KD - 1))
                nc.vector.tensor_tensor(orow[:, n * 500:(n + 1) * 500], hp[:, 0:500],
                                        orow[:, n * 500:(n + 1) * 500], op=OP.add)
            nc.sync.dma_start(out_d.ap(), orow)

    nc.compile()
    return nc


def _tile_w(w):
    """(K*128, C) fp32 -> (128, K*C) bf16 tiled: out[p, k*C+c] = w[k*128+p, c]."""
    k = w.shape[0] // 128
    c = w.shape[1]
    return np.ascontiguousarray(
        w.reshape(k, 128, c).transpose(1, 0, 2).reshape(128, k * c).astype(BFNP))


def _vcol(v):
    """(K*128,) -> (128, K): out[p, k] = v[k*128+p]."""
    k = v.shape[0] // 128
    return v.reshape(k, 128).T


def make_in_maps(inputs):
    f = {n: np.asarray(inputs[n], dtype=np.float32) for n in inputs}

    shared = {}
    shared["patchw"] = _tile_w(f["patch_w"])
    shared["qw"] = np.stack([_tile_w(f["q_w"][l]) for l in range(DEPTH)])
    shared["kvw"] = np.stack([_tile_w(f["kv_w"][l]) for l in range(DEPTH)])
    shared["projw"] = np.stack([_tile_w(f["proj_w"][l]) for l in range(DEPTH)])
    shared["fc1w"] = np.stack([_tile_w(f["fc1_w"][l]) for l in range(DEPTH)])
    shared["fc2w"] = np.stack([_tile_w(f["fc2_w"][l]) for l in range(DEPTH)])
    shared["headw"] = _tile_w(f["head_w"])
    shared["headb"] = f["head_b"]

    vecs = np.zeros((DEPTH, 128, NV), np.float32)
    for l in range(DEPTH):
        vecs[l, :, VO_L1W:VO_L1W + 6] = _vcol(f["ln1_w"][l])
        vecs[l, :, VO_L1B:VO_L1B + 6] = _vcol(f["ln1_b"][l])
        vecs[l, :, VO_QB:VO_QB + 6] = _vcol(f["q_b"][l])
        vecs[l, :, VO_KVB] = f["kv_b"][l]
        vecs[l, :, VO_PB:VO_PB + 6] = _vcol(f["proj_b"][l])
        vecs[l, :, VO_L2W:VO_L2W + 6] = _vcol(f["ln2_w"][l])
        vecs[l, :, VO_L2B:VO_L2B + 6] = _vcol(f["ln2_b"][l])
        vecs[l, :, VO_F1B:VO_F1B + 24] = _vcol(f["fc1_b"][l])
        vecs[l, :, VO_F2B:VO_F2B + 6] = _vcol(f["fc2_b"][l])
    shared["vecs"] = np.ascontiguousarray(vecs)

    normv = np.zeros((128, 12), np.float32)
    normv[:, 0:6] = _vcol(f["norm_w"])
    normv[:, 6:12] = _vcol(f["norm_b"])
    shared["normv"] = normv

    # pos_embed + patch_b / cls folding, transposed token layout
    posb = np.zeros((DIM, TC), np.float32)
    posb[:, 0] = f["cls_token"][0, 0] + f["pos_embed"][0, 0]
    posb[:, 1:T] = (f["pos_embed"][0, 1:T] + f["patch_b"][None, :]).T
    shared["posbt"] = np.ascontiguousarray(
        posb.reshape(KD, 128, TC).transpose(1, 0, 2).reshape(128, KD * TC))

    shared["_ones"] = np.ones((128,), np.float32)
    shared["_zeros"] = np.zeros((KD * TR,), np.float32)

    # per-core im2col (transposed): xpt[(c,a,b), 1 + i*14 + j]
    HG = IMG // P
    x = np.asarray(inputs["x"], dtype=np.float32)
    maps = []
    for b in range(B):
        xp = x[b].reshape(CIN, HG, P, HG, P).transpose(0, 2, 4, 1, 3)
        xp = xp.reshape(DIM, NPATCH)
        xt = np.zeros((DIM, TC), np.float32)
        xt[:, 1:T] = xp
        xt = xt.reshape(KD, 128, TC).transpose(1, 0, 2).reshape(128, KD * TC)
        maps.append(dict(shared, xpt=np.ascontiguousarray(xt.astype(BFNP))))
    return maps


def kernel(**inputs):
    if "nc" not in _CACHED:
        _CACHED["nc"] = build_module()
    nc = _CACHED["nc"]
    res = run_bass_kernel_spmd(nc, make_in_maps(inputs), core_ids=list(range(B)))
    return np.concatenate([res.results[b]["out"] for b in range(B)], axis=0)


# revision 26
# speedup vs baseline: 2.0157x; 1.0532x over previous
"""MoDA Vision Transformer forward pass on 8 Trainium2 NeuronCores.

Sharding: pure data-parallel over batch (B=8 -> 1 image per core, weights
replicated, no collectives).

v2 design (bf16 compute, fp32 residual):
- All weights pre-tiled + cast to bf16 on HOST into exactly the SBUF layout
  [p, k, c], so every weight DMA is one contiguous chunk per partition
  (128 descriptors per load instead of ~10k strided ones).
- Residual stream hT stays fp32 (f32r) padded to 256 token cols so the
  LayerNorm sum matmuls run at full f32r rate; all other activations are
  bf16 at 208 token cols (bf16 matmuls are full rate at any width).
- im2col + pos_embed/cls/bias folding done on host; patch embed is a
  plain linear directly into the transposed residual layout.
- Attention (GQA kv_heads=1, depth-KV cache): q pieces of (1024,1024,316)
  flattened (g,t) queries; per kpos-block one 1024-wide exp on ACT
  (amortizes the 352-cycle ACTIVATE overhead); denominators via a ones
  column appended to V; softmax normalize with DVE reciprocal.
- PSUM: 3x [128,1024] "big" slots (6 banks) + 2x [1-128,512] "row" slots
  (2 banks) = exactly 8 banks.
- Next layer's weights are prefetched at the top of each layer body on the
  sync HWDGE ring; small strip DMAs ride the scalar HWDGE ring.
"""

import numpy as np
import ml_dtypes

import concourse.bass as bass
import concourse.mybir as mybir
import concourse.tile as tile
from concourse import bacc
from concourse.bass_utils import run_bass_kernel_spmd
from concourse.masks import make_identity

F32 = mybir.dt.float32
F32R = mybir.dt.float32r
BF16 = mybir.dt.bfloat16
I32 = mybir.dt.int32
AF = mybir.ActivationFunctionType
OP = mybir.AluOpType
BFNP = ml_dtypes.bfloat16

B, CIN, IMG, P = 8, 3, 224, 16
DIM, DEPTH, NH, NKV = 768, 12, 12, 1
HD = DIM // NH              # 64
G = NH // NKV               # 12
NPATCH = (IMG // P) ** 2    # 196
T = NPATCH + 1              # 197
TC = 208                    # bf16 activation token cols (197 padded)
TR = 256                    # fp32 residual token cols (f32r full-rate >=256)
KD = DIM // 128             # 6
MLP = 4 * DIM               # 3072
MD = MLP // 128             # 24
NQ = G * T                  # 2364
NQP = 2368                  # padded
NKBLK = (DEPTH * T + 127) // 128  # 19
VW = 130                    # V block width (64 V + 1 ones + 63 zero + pad)
SCALE = HD ** -0.5
EPS = 1e-6
NCLS = 1000
QP_ = [(0, 1024), (1024, 1024), (2048, NQ - 2048)]  # q pieces

# packed per-layer vector columns (fp32): [128, NV]
VO_L1W, VO_L1B, VO_QB, VO_KVB, VO_PB = 0, 6, 12, 18, 19
VO_L2W, VO_L2B, VO_F1B, VO_F2B, NV = 25, 31, 37, 61, 67

_CACHED = {}


def build_module():
    nc = bacc.Bacc("TRN2", target_bir_lowering=False, debug=False, num_devices=8)

    d = {}
    def din(name, shape, dt_):
        d[name] = nc.dram_tensor(name, shape, dt_, kind="ExternalInput")

    din("xpt", (128, KD * TC), BF16)
    din("posbt", (128, KD * TC), F32)
    din("patchw", (128, KD * DIM), BF16)
    din("qw", (DEPTH, 128, KD * DIM), BF16)
    din("kvw", (DEPTH, 128, KD * 2 * HD), BF16)
    din("projw", (DEPTH, 128, KD * DIM), BF16)
    din("fc1w", (DEPTH, 128, KD * MLP), BF16)
    din("fc2w", (DEPTH, 128, MD * DIM), BF16)
    din("vecs", (DEPTH, 128, NV), F32)
    din("normv", (128, 12), F32)
    din("headw", (128, KD * NCLS), BF16)
    din("headb", (NCLS,), F32)
    din("_ones", (128,), F32R)
    din("_zeros", (KD * TR,), F32R)
    out_d = nc.dram_tensor("out", (1, NCLS), F32, kind="ExternalOutput")

    with tile.TileContext(nc) as tc:
        with (
            tc.tile_pool(name="persist", bufs=1) as persist,
            tc.tile_pool(name="wq", bufs=2) as wq,        # q/kv/proj + vecs
            tc.tile_pool(name="wsl", bufs=10) as wsl,     # fc1/fc2 quarter slabs
            tc.tile_pool(name="tmp", bufs=2) as tmp,      # transient sbuf
            tc.tile_pool(name="ps", bufs=1, space="PSUM") as ps,
        ):
            # ---------------- persistent state ----------------
            hT = persist.tile([128, KD, TR], F32R)          # residual ^T (fp32)
            sqT = persist.tile([128, KD, TR], BF16)         # squares scratch
            hnT = persist.tile([128, KD, TC], BF16)         # LN output ^T
            oT = persist.tile([128, KD, TC], BF16)          # attn out ^T
            g1T = persist.tile([128, MD, TC], BF16)         # gelu(fc1) ^T
            KC = persist.tile([128, NQP], BF16)             # K cache ^T (x2 dup)
            VC = persist.tile([128, NKBLK, VW], BF16)       # V cache + ones col
            qpt = persist.tile([128, NQP], BF16)            # Q'^T (g,t) (x2 dup)
            otn = persist.tile([64, NQP], BF16)             # normalized O'^T
            ident = persist.tile([128, 128], F32)
            onec = persist.tile([128, 1], F32R)             # 1.0 col (LN sum lhsT)
            onecb = persist.tile([128, 1], BF16)
            oner = persist.tile([1, 128], F32R)             # 1.0 row (bcast lhsT)
            orow = persist.tile([1, NCLS], F32)

            nc.gpsimd.dma_start(
                hT, d["_zeros"].ap().rearrange("(o c) -> o c", o=1)
                .to_broadcast([128, KD * TR]))
            nc.vector.memset(oT, 0.0)
            nc.vector.memset(VC, 0.0)
            make_identity(nc, ident)
            nc.sync.dma_start(onec, d["_ones"].ap().rearrange("(p o) -> p o", o=1))
            nc.sync.dma_start(oner, d["_ones"].ap().rearrange("(o p) -> o p", o=1))
            nc.vector.memset(onecb, 1.0)
            nc.vector.memset(KC[64:128, :], 0.0)
            nc.vector.memset(qpt[64:128, :], 0.0)
            nc.vector.memset(VC[:, :, HD:HD + 2], 1.0)

            # ---------------- weight loading helpers ----------------
            def load_qkvp(l):
                v = wq.tile([128, NV], F32, name="vecs", tag="vecs")
                nc.sync.dma_start(v, d["vecs"].ap()[l])
                qw_ = wq.tile([128, KD, DIM], BF16, name="qw", tag="qw")
                nc.sync.dma_start(qw_, d["qw"].ap()[l].rearrange("p (k c) -> p k c", k=KD))
                kvw_ = wq.tile([128, KD, 2 * HD], BF16, name="kvw", tag="kvw")
                nc.sync.dma_start(kvw_, d["kvw"].ap()[l].rearrange("p (k c) -> p k c", k=KD))
                ow_ = wq.tile([128, KD, DIM], BF16, name="ow", tag="ow")
                nc.sync.dma_start(ow_, d["projw"].ap()[l].rearrange("p (k c) -> p k c", k=KD))
                return v, qw_, kvw_, ow_

            def load_slabs(l):
                f1r = d["fc1w"].ap()[l].rearrange("p (k c) -> p k c", k=KD)
                f2r = d["fc2w"].ap()[l].rearrange("p (k c) -> p k c", k=MD)
                f1q, f2q = [], []
                for i in range(4):  # fc1 quarter: m-tiles 6i/4.. (768 cols each)
                    s = wsl.tile([128, KD, MLP // 4], BF16, name="f1q", tag="slab")
                    nc.sync.dma_start(s, f1r[:, :, i * (MLP // 4):(i + 1) * (MLP // 4)])
                    f1q.append(s)
                for i in range(4):  # fc2 quarter: k-tiles 6i..6i+5 (full 768 cols)
                    s = wsl.tile([128, KD, DIM], BF16, name="f2q", tag="slab")
                    nc.sync.dma_start(s, f2r[:, i * KD:(i + 1) * KD, :])
                    f2q.append(s)
                return f1q, f2q

            # ---------------- layernorm ----------------
            def ln(dst, wb, wo, bo):
                """LN over d of hT -> dst[128, KD, TC] (bf16).

                wb: [128, NV]-style tile; wo/bo: col offsets of gamma/beta."""
                nc.vector.tensor_tensor(sqT, hT, hT, op=OP.mult)
                ssum = ps.tile([1, 512], F32, name="ssum", tag="row", bufs=2)
                ssq = ps.tile([1, 512], F32, name="ssq", tag="row", bufs=2)
                for k in range(KD):
                    nc.tensor.matmul(ssum[:, 0:TR], onec, hT[:, k, :],
                                     start=(k == 0), stop=(k == KD - 1))
                for k in range(KD):
                    nc.tensor.matmul(ssq[:, 0:TR], onecb, sqT[:, k, :],
                                     start=(k == 0), stop=(k == KD - 1))
                mean = tmp.tile([1, TC], F32, name="mean", tag="mean", bufs=1)
                nc.vector.tensor_scalar(out=mean, in0=ssum[:, 0:TC],
                                        scalar1=1.0 / DIM, scalar2=None, op0=OP.mult)
                m2 = tmp.tile([1, TC], F32, name="m2", tag="m2", bufs=1)
                nc.vector.tensor_tensor(m2, mean, mean, op=OP.mult)
                var = tmp.tile([1, TC], F32, name="var", tag="var", bufs=1)
                nc.vector.scalar_tensor_tensor(
                    out=var, in0=ssq[:, 0:TC], scalar=1.0 / DIM,
                    in1=m2, op0=OP.mult, op1=OP.subtract)
                ve = tmp.tile([1, TC], F32, name="ve", tag="ve", bufs=1)
                nc.vector.tensor_scalar(out=ve, in0=var, scalar1=EPS,
                                        scalar2=None, op0=OP.add)
                sd = tmp.tile([1, TC], I32, name="sd", tag="sd", bufs=1)
                nc.vector.tensor_scalar(out=sd, in0=ve.bitcast(I32), scalar1=1,
                                        scalar2=None, op0=OP.logical_shift_right)
                nc.vector.tensor_scalar(out=sd, in0=sd, scalar1=-1,
                                        scalar2=0x5F3759DF, op0=OP.mult, op1=OP.add)
                y0 = sd.bitcast(F32)
                t_ = tmp.tile([1, TC], F32, name="nrT", tag="nrT", bufs=1)
                nc.vector.tensor_tensor(t_, y0, y0, op=OP.mult)
                nc.vector.tensor_tensor(t_, t_, ve, op=OP.mult)
                nc.vector.tensor_scalar(out=t_, in0=t_, scalar1=-0.5, scalar2=1.5,
                                        op0=OP.mult, op1=OP.add)
                rstd = tmp.tile([1, TC], F32R, name="rstd", tag="rstd", bufs=1)
                nc.vector.tensor_tensor(rstd, y0, t_, op=OP.mult)
                mr = tmp.tile([1, TC], F32R, name="mr", tag="mr", bufs=1)
                nc.vector.tensor_tensor(mr, mean, rstd, op=OP.mult)
                for _ in range(8):
                    dmy = ps.tile([128, 512], F32, name="dmyl", tag="row", bufs=2)
                    nc.tensor.matmul(dmy, KC[:, 0:128], qpt[:, 0:512],
                                     start=True, stop=True)
                rstd_b = ps.tile([128, TC], F32, name="rstd_b", tag="row", bufs=2)
                mr_b = ps.tile([128, TC], F32, name="mr_b", tag="row", bufs=2)
                nc.tensor.matmul(rstd_b, oner, rstd, start=True, stop=True)
                nc.tensor.matmul(mr_b, oner, mr, start=True, stop=True)
                for _ in range(6):
                    dmy = ps.tile([128, 512], F32, name="dmyl", tag="row", bufs=2)
                    nc.tensor.matmul(dmy, KC[:, 0:128], qpt[:, 0:512],
                                     start=True, stop=True)
                for k in range(KD):
                    t1 = tmp.tile([128, TC], F32, name="lnt", tag="lnt", bufs=2)
                    nc.vector.tensor_tensor(t1, hT[:, k, 0:TC], rstd_b, op=OP.mult)
                    nc.vector.tensor_tensor(t1, t1, mr_b, op=OP.subtract)
                    nc.vector.tensor_scalar(
                        out=dst[:, k, :], in0=t1,
                        scalar1=wb[:, wo + k:wo + k + 1],
                        scalar2=wb[:, bo + k:bo + k + 1],
                        op0=OP.mult, op1=OP.add)

            # ---------------- prologue: patch embed ----------------
            xpt = tmp.tile([128, KD, TC], BF16, name="xpt", tag="pt", bufs=2)
            nc.sync.dma_start(xpt, d["xpt"].ap().rearrange("p (k c) -> p k c", k=KD))
            posbt = wsl.tile([128, KD, TC], F32, name="posbt", tag="slab")
            nc.sync.dma_start(posbt, d["posbt"].ap().rearrange("p (k c) -> p k c", k=KD))
            pw_sb = wq.tile([128, KD, DIM], BF16, name="pw_sb", tag="qw")
            nc.sync.dma_start(pw_sb, d["patchw"].ap().rearrange("p (k c) -> p k c", k=KD))
            vecs0 = load_qkvp(0)
            slabs0 = load_slabs(0)

            for m in range(KD):
                pp = ps.tile([128, 1024], F32, name="pp", tag="big", bufs=3)
                for k in range(KD):
                    nc.tensor.matmul(pp[:, 0:TC], pw_sb[:, k, m * 128:(m + 1) * 128],
                                     xpt[:, k, :], start=(k == 0), stop=(k == KD - 1))
                nc.vector.tensor_tensor(hT[:, m, 0:TC], pp[:, 0:TC],
                                        posbt[:, m, :], op=OP.add)

            # ---------------- transformer layers ----------------
            lw = (vecs0, slabs0)
            for l in range(DEPTH):
                (vv, qw_sb, kvw_sb, ow_sb), (f1q, f2q) = lw
                if l + 1 < DEPTH:
                    nxt = (load_qkvp(l + 1), load_slabs(l + 1))

                ln(hnT, vv, VO_L1W, VO_L1B)

                # ---- Q projection -> qpt strips ----
                for m in range(KD):
                    qp = ps.tile([128, 1024], F32, name="qp", tag="big", bufs=3)
                    for k in range(KD):
                        nc.tensor.matmul(qp[:, 0:TC], qw_sb[:, k, m * 128:(m + 1) * 128],
                                         hnT[:, k, :], start=(k == 0), stop=(k == KD - 1))
                    nc.vector.tensor_scalar(
                        out=qpt[0:64, (2 * m) * T:(2 * m) * T + T],
                        in0=qp[0:64, 0:T],
                        scalar1=vv[0:64, VO_QB + m:VO_QB + m + 1], scalar2=None,
                        op0=OP.add)
                    nc.vector.tensor_scalar(
                        out=qpt[0:64, (2 * m + 1) * T:(2 * m + 1) * T + T],
                        in0=qp[64:128, 0:T],
                        scalar1=vv[64:128, VO_QB + m:VO_QB + m + 1], scalar2=None,
                        op0=OP.add)

                # ---- KV projection; append K^T and V to caches ----
                kvp = ps.tile([128, 1024], F32, name="kvp", tag="big", bufs=3)
                for k in range(KD):
                    nc.tensor.matmul(kvp[:, 0:TC], kvw_sb[:, k, :], hnT[:, k, :],
                                     start=(k == 0), stop=(k == KD - 1))
                nc.vector.tensor_scalar(
                    out=KC[0:64, l * T:l * T + T], in0=kvp[0:64, 0:T],
                    scalar1=vv[0:64, VO_KVB:VO_KVB + 1], scalar2=None, op0=OP.add)
                vsb = tmp.tile([128, TC], F32, name="vsb", tag="vsb", bufs=1)
                nc.vector.tensor_scalar(
                    out=vsb[64:128, :], in0=kvp[64:128, 0:TC],
                    scalar1=vv[64:128, VO_KVB:VO_KVB + 1], scalar2=None, op0=OP.add)
                for tc_i, tsz in ((0, 128), (1, 69)):
                    vtp = ps.tile([128, 512], F32, name="vtp", tag="row", bufs=2)
                    nc.tensor.matmul(vtp[0:tsz, 0:HD],
                                     vsb[64:128, tc_i * 128:tc_i * 128 + tsz],
                                     ident[64:128, 64:64 + HD], is_transpose=True,
                                     start=True, stop=True)
                    vts = tmp.tile([128, HD], BF16, name="vts", tag="vts", bufs=2)
                    nc.vector.tensor_copy(vts[0:tsz], vtp[0:tsz, 0:HD])
                    t0 = 0
                    while t0 < tsz:
                        kpos = l * T + tc_i * 128 + t0
                        blk, off = kpos // 128, kpos % 128
                        cnt = min(tsz - t0, 128 - off)
                        nc.scalar.dma_start(
                            VC[off:off + cnt, blk, 0:HD],
                            vts[t0:t0 + cnt, :])
                        t0 += cnt

                # ---- attention ----
                Lk = (l + 1) * T
                nkt = (Lk + 127) // 128
                for qoff, qsz in QP_:
                    nh = (qsz + 511) // 512
                    ot = ps.tile([128, 1024], F32, name="ot", tag="big", bufs=3)
                    for c in range(nkt):
                        ksz = min(128, Lk - c * 128)
                        st = ps.tile([128, 1024], F32, name="st", tag="big", bufs=3)
                        for h in range(nh):
                            cw = min(512, qsz - h * 512)
                            nc.tensor.matmul(
                                st[0:ksz, h * 512:h * 512 + cw],
                                KC[:, c * 128:c * 128 + ksz],
                                qpt[:, qoff + h * 512:qoff + h * 512 + cw],
                                start=True, stop=True)
                        dmy = ps.tile([128, 512], F32, name="dmy", tag="row",
                                      bufs=2)
                        nc.tensor.matmul(dmy, KC[:, 0:128], qpt[:, 0:512],
                                         start=True, stop=True)
                        pt = tmp.tile([128, 1024], BF16, name="pt", tag="pt", bufs=2)
                        nc.scalar.activation(pt[0:ksz, 0:qsz], st[0:ksz, 0:qsz],
                                             AF.Exp, scale=SCALE)
                        for h in range(nh):
                            cw = min(512, qsz - h * 512)
                            nc.tensor.matmul(
                                ot[:, h * 512:h * 512 + cw],
                                VC[0:ksz, c, 0:128],
                                pt[0:ksz, h * 512:h * 512 + cw],
                                start=(c == 0), stop=(c == nkt - 1))
                    # evacuate ot fast (frees the PSUM slot for the next
                    # piece), then normalize off the critical path
                    ots = tmp.tile([65, 1024], F32, name="ots", tag="ots", bufs=2)
                    nc.vector.tensor_copy(ots[:, 0:qsz], ot[0:65, 0:qsz])
                    for h in range(nh):
                        cw = min(512, qsz - h * 512)
                        denr = tmp.tile([1, 512], F32R, name="denr", tag="rec", bufs=2)
                        nc.vector.tensor_copy(denr[:, 0:cw],
                                              ots[64:65, h * 512:h * 512 + cw])
                        rbp = ps.tile([64, 512], F32, name="rbp", tag="row", bufs=2)
                        nc.tensor.matmul(rbp[:, 0:cw], oner[:, 0:64], denr[:, 0:cw],
                                         start=True, stop=True)
                        rb = tmp.tile([64, 512], F32, name="rb", tag="rb", bufs=2)
                        nc.vector.reciprocal_approx_fast(rb[:, 0:cw], rbp[:, 0:cw])
                        nc.vector.tensor_tensor(
                            otn[:, qoff + h * 512:qoff + h * 512 + cw],
                            ots[0:64, h * 512:h * 512 + cw], rb[:, 0:cw],
                            op=OP.mult)

                # ---- reshape O'T (g,t) -> oT [d, t] ----
                for g in range(G):
                    j, half = g // 2, g % 2
                    nc.vector.tensor_copy(oT[64 * half:64 * half + 64, j, 0:T],
                                          otn[:, g * T:g * T + T])

                # ---- output projection + residual ----
                for _ in range(6):
                    dmy = ps.tile([128, 512], F32, name="dmyp", tag="row", bufs=2)
                    nc.tensor.matmul(dmy, KC[:, 0:128], qpt[:, 0:512],
                                     start=True, stop=True)
                for m in range(KD):
                    op_ = ps.tile([128, 1024], F32, name="prp", tag="big", bufs=3)
                    for k in range(KD):
                        nc.tensor.matmul(op_[:, 0:TC], ow_sb[:, k, m * 128:(m + 1) * 128],
                                         oT[:, k, :], start=(k == 0), stop=(k == KD - 1))
                    nc.vector.scalar_tensor_tensor(
                        out=hT[:, m, 0:T], in0=op_[:, 0:T],
                        scalar=vv[:, VO_PB + m:VO_PB + m + 1],
                        in1=hT[:, m, 0:T], op0=OP.add, op1=OP.add)

                # ---- MLP ----
                ln(hnT, vv, VO_L2W, VO_L2B)
                for m in range(MD):
                    f1s = f1q[m // 6]
                    mi = m % 6
                    fp = ps.tile([128, 1024], F32, name="fp", tag="big", bufs=3)
                    for k in range(KD):
                        nc.tensor.matmul(fp[:, 0:TC], f1s[:, k, mi * 128:(mi + 1) * 128],
                                         hnT[:, k, :], start=(k == 0), stop=(k == KD - 1))
                    nc.scalar.activation(g1T[:, m, :], fp[:, 0:TC], AF.Gelu,
                                         bias=vv[:, VO_F1B + m:VO_F1B + m + 1])
                for m in range(KD):
                    f2p = ps.tile([128, 1024], F32, name="f2p", tag="big", bufs=3)
                    for k in range(MD):
                        f2s = f2q[k // 6]
                        nc.tensor.matmul(f2p[:, 0:TC],
                                         f2s[:, k % 6, m * 128:(m + 1) * 128],
                                         g1T[:, k, :], start=(k == 0), stop=(k == MD - 1))
                    nc.vector.scalar_tensor_tensor(
                        out=hT[:, m, 0:T], in0=f2p[:, 0:T],
                        scalar=vv[:, VO_F2B + m:VO_F2B + m + 1],
                        in1=hT[:, m, 0:T], op0=OP.add, op1=OP.add)

                if l + 1 < DEPTH:
                    lw = nxt

            # ---------------- final LN + head ----------------
            nv = persist.tile([128, 12], F32)
            nc.sync.dma_start(nv, d["normv"].ap())
            nc.sync.dma_start(orow, d["headb"].ap().rearrange("(o c) -> o c", o=1))
            ln(hnT, nv, 0, 6)
            hwr = d["headw"].ap().rearrange("p (k c) -> p k c", k=KD)
            for n in range(2):
                hw_c = wsl.tile([128, KD, 500], BF16, name="hw_c", tag="slab")
                nc.sync.dma_start(hw_c, hwr[:, :, n * 500:(n + 1) * 500])
                hp = ps.tile([1, 512], F32, name="hp", tag="row", bufs=2)
                for k in range(KD):
                    nc.tensor.matmul(hp[:, 0:500], hnT[:, k, 0:1], hw_c[:, k, :],
                                     start=(k == 0), stop=(k == KD - 1))
                nc.vector.tensor_tensor(orow[:, n * 500:(n + 1) * 500], hp[:, 0:500],
                                        orow[:, n * 500:(n + 1) * 500], op=OP.add)
            nc.sync.dma_start(out_d.ap(), orow)

    nc.compile()
    return nc


def _tile_w(w):
    """(K*128, C) fp32 -> (128, K*C) bf16 tiled: out[p, k*C+c] = w[k*128+p, c]."""
    k = w.shape[0] // 128
    c = w.shape[1]
    return np.ascontiguousarray(
        w.reshape(k, 128, c).transpose(1, 0, 2).reshape(128, k * c).astype(BFNP))


def _vcol(v):
    """(K*128,) -> (128, K): out[p, k] = v[k*128+p]."""
    k = v.shape[0] // 128
    return v.reshape(k, 128).T


def make_in_maps(inputs):
    f = {n: np.asarray(inputs[n], dtype=np.float32) for n in inputs}

    shared = {}
    shared["patchw"] = _tile_w(f["patch_w"])
    shared["qw"] = np.stack([_tile_w(f["q_w"][l]) for l in range(DEPTH)])
    shared["kvw"] = np.stack([_tile_w(f["kv_w"][l]) for l in range(DEPTH)])
    shared["projw"] = np.stack([_tile_w(f["proj_w"][l]) for l in range(DEPTH)])
    shared["fc1w"] = np.stack([_tile_w(f["fc1_w"][l]) for l in range(DEPTH)])
    shared["fc2w"] = np.stack([_tile_w(f["fc2_w"][l]) for l in range(DEPTH)])
    shared["headw"] = _tile_w(f["head_w"])
    shared["headb"] = f["head_b"]

    vecs = np.zeros((DEPTH, 128, NV), np.float32)
    for l in range(DEPTH):
        vecs[l, :, VO_L1W:VO_L1W + 6] = _vcol(f["ln1_w"][l])
        vecs[l, :, VO_L1B:VO_L1B + 6] = _vcol(f["ln1_b"][l])
        vecs[l, :, VO_QB:VO_QB + 6] = _vcol(f["q_b"][l])
        vecs[l, :, VO_KVB] = f["kv_b"][l]
        vecs[l, :, VO_PB:VO_PB + 6] = _vcol(f["proj_b"][l])
        vecs[l, :, VO_L2W:VO_L2W + 6] = _vcol(f["ln2_w"][l])
        vecs[l, :, VO_L2B:VO_L2B + 6] = _vcol(f["ln2_b"][l])
        vecs[l, :, VO_F1B:VO_F1B + 24] = _vcol(f["fc1_b"][l])
        vecs[l, :, VO_F2B:VO_F2B + 6] = _vcol(f["fc2_b"][l])
    shared["vecs"] = np.ascontiguousarray(vecs)

    normv = np.zeros((128, 12), np.float32)
    normv[:, 0:6] = _vcol(f["norm_w"])
    normv[:, 6:12] = _vcol(f["norm_b"])
    shared["normv"] = normv

    # pos_embed + patch_b / cls folding, transposed token layout
    posb = np.zeros((DIM, TC), np.float32)
    posb[:, 0] = f["cls_token"][0, 0] + f["pos_embed"][0, 0]
    posb[:, 1:T] = (f["pos_embed"][0, 1:T] + f["patch_b"][None, :]).T
    shared["posbt"] = np.ascontiguousarray(
        posb.reshape(KD, 128, TC).transpose(1, 0, 2).reshape(128, KD * TC))

    shared["_ones"] = np.ones((128,), np.float32)
    shared["_zeros"] = np.zeros((KD * TR,), np.float32)

    # per-core im2col (transposed): xpt[(c,a,b), 1 + i*14 + j]
    HG = IMG // P
    x = np.asarray(inputs["x"], dtype=np.float32)
    maps = []
    for b in range(B):
        xp = x[b].reshape(CIN, HG, P, HG, P).transpose(0, 2, 4, 1, 3)
        xp = xp.reshape(DIM, NPATCH)
        xt = np.zeros((DIM, TC), np.float32)
        xt[:, 1:T] = xp
        xt = xt.reshape(KD, 128, TC).transpose(1, 0, 2).reshape(128, KD * TC)
        maps.append(dict(shared, xpt=np.ascontiguousarray(xt.astype(BFNP))))
    return maps


def kernel(**inputs):
    if "nc" not in _CACHED:
        _CACHED["nc"] = build_module()
    nc = _CACHED["nc"]
    res = run_bass_kernel_spmd(nc, make_in_maps(inputs), core_ids=list(range(B)))
    return np.concatenate([res.results[b]["out"] for b in range(B)], axis=0)
